# revision 22
# baseline (speedup 1.0000x reference)
"""Modulated deformable conv (DCNv2) Bass kernel for Trainium2, 8 NeuronCores.

Sharding: data-parallel over batch x row-halves; core i handles sample i//2,
output rows 64*(i%2) .. 64*(i%2)+63. No cross-core communication.

v3: multi-queue SWDGE gathers (4 Q7 pairs in parallel), dx-packed offset
conv (81-row lhsT), xbar DMA transposes for both om layouts, and a
2-op-per-tap combine (weight-mult + corner reduce) on a [c, corner] token.

Per-core pipeline:
  B. PE: offset/mask conv (128ch -> 27ch, 3x3) as 3 dy-packed matmuls
     (81-row lhsT = 3 dx shifts), PSUM acc; DVE sums the column-shifted
     dx blocks + bias -> om (bf16, 32 rows).
  C. Xbar DMA transpose: om -> omT natural (position-on-partition) and
     omTw wrapped (gather token order), both [*, 32]-strided bf16.
  D. DVE: fractional bilinear weights with mask + out-of-bounds validity
     folded into 4 corner weights wq[p, g, k, j] (bf16), plus patch-anchor
     gather indices (int32 -> wrapped int16).
  F. Indirect DMA gather (SWDGE): 512B tokens = 64ch x 2x2 pixel patch
     ([c, corner] layout) from a host-prebuilt patch table in HBM; one
     call per (quarter, tap-pair) = 20 calls spread over 4 SWDGE queues
     (4 Q7 descriptor-generator pairs run concurrently).
  G. DVE: per-tap corner-weight multiply (1 op) + corner reduce (1 op).
  H. PE: paired-tap transposes ([128,128] bf16) + main conv with 128-deep
     contraction (2 taps x 64ch); bias added during PSUM->SBUF copy (ACT).
"""
import sys

for _p in ("/opt/trn_rl_repo", "/root/.axon_site/_ro/trn_rl_repo"):
    if _p not in sys.path:
        sys.path.append(_p)

import numpy as np
import ml_dtypes

import concourse.bacc as bacc
import concourse.bass as bass
import concourse.mybir as mybir
import concourse.tile as tile
from concourse.masks import make_identity

F32 = mybir.dt.float32
BF16 = mybir.dt.bfloat16
I32 = mybir.dt.int32
ALU = mybir.AluOpType
ACTF = mybir.ActivationFunctionType
AXL = mybir.AxisListType
BF = ml_dtypes.bfloat16

B, C, H, W = 4, 64, 128, 128
O, K2 = 64, 9
NCORES = 8
R = H // 2             # output rows per core
PW = 130               # patch-table width (anchors -1..128)
NQ = 4                 # quarters of the per-core position space
GQ = 16                # row-chunks (=output rows) per quarter
SH = 4.0               # +4 domain shift so floor domain is positive
KOFF = -(3.0 * PW + 3.0)   # anchor idx = (y0s-3)*130 + (x0s-3)


DEBUG = False


def rr(t, spec, **kw):
    return t[:].rearrange(spec, **kw)


def build_program():
    nc = bacc.Bacc("TRN2", num_swdge_queues=4)
    d_x2 = nc.dram_tensor("x2", [128, 66 * PW], BF16, kind="ExternalInput")
    d_patch = nc.dram_tensor("patch", [PW * PW, 256], BF16, kind="ExternalInput")
    d_base = nc.dram_tensor("base", [128, 64 * 18], BF16, kind="ExternalInput")
    d_basew = nc.dram_tensor("basew", [128, 16 * 4 * 18], BF16, kind="ExternalInput")
    d_womt = nc.dram_tensor("womt", [128, 3 * 96], BF16, kind="ExternalInput")
    d_bomt = nc.dram_tensor("bomt", [27, 1], F32, kind="ExternalInput")
    d_wm2 = nc.dram_tensor("wm2", [128, 4 * 64], BF16, kind="ExternalInput")
    d_wms = nc.dram_tensor("wms", [64, 64], BF16, kind="ExternalInput")
    d_biast = nc.dram_tensor("biast", [64, 1], F32, kind="ExternalInput")
    d_out = nc.dram_tensor("out", [64, R * W], F32, kind="ExternalOutput")
    if DEBUG:
        d_dbg_om = nc.dram_tensor("dbg_om", [32, R * W], BF16,
                                  kind="ExternalOutput")
        d_dbg_omT = nc.dram_tensor("dbg_omT", [128, 64 * 32], BF16,
                                   kind="ExternalOutput")
        d_dbg_omTw = nc.dram_tensor("dbg_omTw", [128, 16 * 4 * 32], BF16,
                                    kind="ExternalOutput")
        d_dbg_wq = nc.dram_tensor("dbg_wq", [128, 64 * 9 * 4], BF16,
                                  kind="ExternalOutput")
        d_dbg_idxw = nc.dram_tensor("dbg_idxw", [128, 9 * 4 * 128],
                                    mybir.dt.int16, kind="ExternalOutput")

    with tile.TileContext(nc) as tc:
        with (
            tc.tile_pool(name="consts", bufs=1) as cpool,
        ):
            identb = cpool.tile([128, 128], BF16)
            make_identity(nc, identb[:])
            identf = cpool.tile([128, 128], F32)
            make_identity(nc, identf[:])
            womt = cpool.tile([128, 3 * 96], BF16)
            bomt = cpool.tile([27, 1], F32)
            base = cpool.tile([128, 64 * 18], BF16)
            basew = cpool.tile([128, 16 * 4 * 18], BF16)
            wm2 = cpool.tile([128, 4 * 64], BF16)
            wms = cpool.tile([64, 64], BF16)
            biast = cpool.tile([64, 1], F32)
            for sb, dr in ((womt, d_womt), (bomt, d_bomt), (base, d_base),
                           (basew, d_basew), (wm2, d_wm2), (wms, d_wms),
                           (biast, d_biast)):
                nc.sync.dma_start(sb[:], dr[:])

            # corner weights wq[p, g64, k9, j4] bf16 + gather indices
            wq = cpool.tile([128, 64 * 9 * 4], BF16)
            idxw = cpool.tile([128, 9 * 4 * 128], mybir.dt.int16)

            with (
                tc.tile_pool(name="mid", bufs=1) as midpool,
                tc.tile_pool(name="tmp", bufs=1) as tpool,
                tc.tile_pool(name="pso", bufs=2, space="PSUM") as ps_om,
                tc.tile_pool(name="pstp", bufs=2, space="PSUM") as ps_tp,
            ):
                omT = midpool.tile([128, 64 * 32], BF16)
                om = midpool.tile([32, R * W], BF16)
                om2 = midpool.tile([32, R * W], BF16)
                omTw = midpool.tile([128, 16 * 4 * 32], BF16)
                omTv = rr(omT, "p (g s) -> p g s", s=32)
                # ---- B: om conv (3 dy-packed matmuls, dx folded on DVE) ----
                x2 = midpool.tile([128, 66 * PW], BF16)
                nc.sync.dma_start(x2[:], d_x2[:])
                x2v = rr(x2, "p (r c) -> p r c", c=PW)
                omv = rr(om, "s (nt c) -> s nt c", c=512)
                om2w = rr(om2, "s (q x) -> s q x", q=16)
                for nt in range(16):
                    ps = ps_om.tile([96, 1024], F32)
                    psf = rr(ps, "p (r x) -> p r x", x=256)
                    for dy in range(3):
                        lhsT = womt[:, dy * 96:(dy + 1) * 96]
                        for r4 in range(4):
                            # start=True clears has_written for the WHOLE
                            # bank: only the first region per bank sets it.
                            nc.tensor.matmul(
                                ps[:, 256 * r4:256 * r4 + 130],
                                lhsT=lhsT,
                                rhs=x2v[:, nt * 4 + dy + r4, :],
                                start=(dy == 0 and r4 % 2 == 0),
                                stop=(dy == 2))
                    t0 = tpool.tile([27, 512], F32, tag="om_t0")
                    t1 = tpool.tile([27, 512], F32, tag="om_t1")
                    nc.vector.tensor_tensor(
                        out=rr(t0, "p (r x) -> p r x", x=128),
                        in0=psf[0:27, :, 0:128],
                        in1=bomt[:, 0:1].to_broadcast([27, 4, 128]), op=ALU.add)
                    nc.vector.tensor_tensor(
                        out=rr(t1, "p (r x) -> p r x", x=128),
                        in0=psf[32:59, :, 1:129],
                        in1=rr(t0, "p (r x) -> p r x", x=128), op=ALU.add)
                    nc.vector.tensor_tensor(
                        out=omv[0:27, nt, :].rearrange("s (r x) -> s r x",
                                                       x=128),
                        in0=psf[64:91, :, 2:130],
                        in1=rr(t1, "p (r x) -> p r x", x=128), op=ALU.add)
                    # wrap-ordered duplicate: om2[s, 512q + xw] = om[s, 16xw + q]
                    nc.vector.tensor_tensor(
                        out=om2w[0:27, :, 32 * nt:32 * nt + 32]
                        .rearrange("s q (r xh) -> s r xh q", xh=8),
                        in0=psf[64:91, :, 2:130]
                        .rearrange("s r (xh q) -> s r xh q", q=16),
                        in1=rr(t1, "p (r xh q) -> p r xh q", xh=8, q=16),
                        op=ALU.add)

                # ---- C: omT natural + omTw wrapped via xbar DMA transpose ----
                nc.sync.dma_start_transpose(omTv, om[:])
                omTwv = rr(omTw, "p (q c s) -> p q c s", q=16, c=4)
                nc.sync.dma_start_transpose(
                    rr(omTw, "p (t s) -> p t s", s=32), om2[:])
                nc.scalar.activation(omTv[:, :, 18:27], omTv[:, :, 18:27],
                                     ACTF.Sigmoid)

                # ---- D: bilinear corner weights + gather indices ----
                basev = rr(base, "p (g s) -> p g s", s=18)

                def v18(t):
                    return rr(t, "p (g s) -> p g s", s=18)

                def v9(t):
                    return rr(t, "p (g s) -> p g s", s=9)

                pypx = tpool.tile([128, 64 * 18], F32)
                nc.vector.tensor_add(out=v18(pypx), in0=omTv[:, :, 0:18],
                                     in1=basev)
                ri32 = tpool.tile([128, 64 * 18], mybir.dt.int32)
                nc.vector.tensor_copy(ri32[:], pypx[:])
                rf32 = tpool.tile([128, 64 * 18], F32)
                nc.vector.tensor_copy(rf32[:], ri32[:])
                gt_ = tpool.tile([128, 64 * 18], F32, tag="ri32")
                nc.vector.tensor_tensor(out=gt_[:], in0=rf32[:], in1=pypx[:],
                                        op=ALU.is_gt)
                flor = tpool.tile([128, 64 * 18], F32)
                nc.vector.tensor_tensor(out=flor[:], in0=rf32[:], in1=gt_[:],
                                        op=ALU.subtract)
                frac = tpool.tile([128, 64 * 18], F32)
                nc.vector.tensor_tensor(out=frac[:], in0=pypx[:], in1=flor[:],
                                        op=ALU.subtract)
                f0c = tpool.tile([128, 64 * 18], F32)
                nc.vector.tensor_scalar(f0c[:], flor[:], SH, 127.0 + SH,
                                        ALU.max, ALU.min)
                v0 = tpool.tile([128, 64 * 18], F32)
                nc.vector.tensor_tensor(out=v0[:], in0=f0c[:], in1=flor[:],
                                        op=ALU.is_equal)
                f1 = tpool.tile([128, 64 * 18], F32)
                nc.vector.tensor_scalar(f1[:], flor[:], 1.0, None, ALU.add)
                f1c = tpool.tile([128, 64 * 18], F32)
                nc.vector.tensor_scalar(f1c[:], f1[:], SH, 127.0 + SH,
                                        ALU.max, ALU.min)
                v1 = tpool.tile([128, 64 * 18], F32)
                nc.vector.tensor_tensor(out=v1[:], in0=f1c[:], in1=f1[:],
                                        op=ALU.is_equal)

                wy, wx = v18(frac)[:, :, 0:9], v18(frac)[:, :, 9:18]
                vy0, vx0 = v18(v0)[:, :, 0:9], v18(v0)[:, :, 9:18]
                vy1, vx1 = v18(v1)[:, :, 0:9], v18(v1)[:, :, 9:18]
                mskf = tpool.tile([128, 64 * 9], F32)
                nc.vector.tensor_copy(mskf[:], omTv[:, :, 18:27])
                msk = v9(mskf)

                a0 = tpool.tile([128, 64 * 9], F32)
                a1 = tpool.tile([128, 64 * 9], F32)
                b0 = tpool.tile([128, 64 * 9], F32)
                b1 = tpool.tile([128, 64 * 9], F32)
                a0v, a1v, b0v, b1v = v9(a0), v9(a1), v9(b0), v9(b1)
                nc.vector.tensor_scalar(a0[:], wy, -1.0, 1.0, ALU.mult, ALU.add)
                nc.vector.tensor_tensor(out=a0v, in0=a0v, in1=vy0, op=ALU.mult)
                nc.vector.tensor_tensor(out=a0v, in0=a0v, in1=msk, op=ALU.mult)
                nc.vector.tensor_tensor(out=a1v, in0=wy, in1=vy1, op=ALU.mult)
                nc.vector.tensor_tensor(out=a1v, in0=a1v, in1=msk, op=ALU.mult)
                nc.vector.tensor_scalar(b0[:], wx, -1.0, 1.0, ALU.mult, ALU.add)
                nc.vector.tensor_tensor(out=b0v, in0=b0v, in1=vx0, op=ALU.mult)
                nc.vector.tensor_tensor(out=b1v, in0=wx, in1=vx1, op=ALU.mult)

                wqv = rr(wq, "p (g k j) -> p g k j", g=64, k=9)
                for j, (ya, xb) in enumerate(((a0v, b0v), (a0v, b1v),
                                              (a1v, b0v), (a1v, b1v))):
                    nc.vector.tensor_tensor(
                        out=wqv[:, :, :, j], in0=ya, in1=xb, op=ALU.mult)

                # ---- D2: wrapped pipeline -> patch anchor indices ----
                def w18(t):
                    return rr(t, "p (q c s) -> p q c s", q=16, c=4, s=18)

                def w9t(t):
                    return rr(t, "p (q c s) -> p q c s", q=16, c=4, s=9)

                pypw = tpool.tile([128, 16 * 4 * 18], F32)
                nc.vector.tensor_add(out=w18(pypw),
                                     in0=omTwv[:, :, :, 0:18],
                                     in1=rr(basew, "p (q c s) -> p q c s",
                                            q=16, c=4))
                ri32w = tpool.tile([128, 16 * 4 * 18], mybir.dt.int32)
                nc.vector.tensor_copy(ri32w[:], pypw[:])
                rf32w = tpool.tile([128, 16 * 4 * 18], F32)
                nc.vector.tensor_copy(rf32w[:], ri32w[:])
                gtw_ = tpool.tile([128, 16 * 4 * 18], F32, tag="ri32w")
                nc.vector.tensor_tensor(out=gtw_[:], in0=rf32w[:], in1=pypw[:],
                                        op=ALU.is_gt)
                florw = tpool.tile([128, 16 * 4 * 18], F32)
                nc.vector.tensor_tensor(out=florw[:], in0=rf32w[:],
                                        in1=gtw_[:], op=ALU.subtract)
                # clamp anchors to [-1, 128] (+SH domain: [3, 132])
                fy0w = tpool.tile([128, 16 * 4 * 9], F32)
                nc.vector.tensor_scalar(w9t(fy0w), w18(florw)[:, :, :, 0:9],
                                        SH - 1.0, 128.0 + SH, ALU.max, ALU.min)
                gxw = tpool.tile([128, 16 * 4 * 9], F32)
                nc.vector.tensor_scalar(w9t(gxw), w18(florw)[:, :, :, 9:18],
                                        SH - 1.0, 128.0 + SH, ALU.max, ALU.min)
                idxt = tpool.tile([128, 16 * 4 * 9], F32)
                # anchor = (y0s-3)*130 + (x0s-3), y0s/x0s in +4 domain
                nc.vector.tensor_scalar(idxt[:], fy0w[:], float(PW), KOFF,
                                        ALU.mult, ALU.add)
                nc.vector.tensor_tensor(out=idxt[:], in0=idxt[:], in1=gxw[:],
                                        op=ALU.add)

                # ---- E: fold indices to wrapped int16 layout [cq][k][128]
                for k in range(K2):
                    srcT = w9t(idxt)
                    for cq in range(4):
                        pv = ps_tp.tile([16, 128], F32, tag="pidx")
                        nc.tensor.transpose(pv[:], srcT[:, :, cq, k],
                                            identf[:, :])
                        off = (cq * 9 + k) * 128
                        nc.vector.tensor_copy(idxw[0:16, off:off + 128],
                                              pv[:])
                for g in range(1, 8):
                    nc.sync.dma_start(idxw[16 * g:16 * (g + 1), :],
                                      idxw[0:16, :])
                if DEBUG:
                    nc.sync.dma_start(d_dbg_om[:], om[:])
                    nc.sync.dma_start(d_dbg_omT[:], omT[:])
                    nc.sync.dma_start(d_dbg_omTw[:], omTw[:])
                    nc.sync.dma_start(d_dbg_wq[:], wq[:])
                    nc.sync.dma_start(d_dbg_idxw[:], idxw[:])

            # ---- F/G/H: gather, combine, transpose, main conv ----
            wqv2 = rr(wq, "p (g k j) -> p g k j", g=64, k=9)
            with (
                tc.tile_pool(name="gat", bufs=4) as gpool,
                tc.tile_pool(name="gat1", bufs=2) as gpool1,
                tc.tile_pool(name="outp", bufs=1) as opool,
                tc.tile_pool(name="comb", bufs=2) as mpool,
                tc.tile_pool(name="pstx", bufs=2, space="PSUM") as ps_tx,
                tc.tile_pool(name="psmain", bufs=1, space="PSUM") as ps_main,
            ):
                out_sb = opool.tile([64, R * W], F32, tag="out_sb")
                reg2k = nc.gpsimd.to_reg(2048)
                reg4k = nc.gpsimd.to_reg(4096)
                for cq in range(NQ):
                    ops = ps_main.tile([64, 2048], F32)
                    for pr in range(5):
                        npair = 2 if pr < 4 else 1
                        nslot = GQ * npair
                        gt = (gpool if npair == 2 else gpool1).tile(
                            [128, nslot * 256], BF16, tag=f"gt{npair}")
                        k0 = 2 * pr
                        ioff = (cq * 9 + k0) * 128
                        nc.gpsimd.dma_gather(
                            rr(gt, "p (i e) -> p i e", e=256),
                            d_patch[:],
                            idxw[:, ioff:ioff + 128 * npair],
                            2048 * npair,
                            reg4k if npair == 2 else reg2k,
                            256,
                            single_packet=False,
                            queue_num=(cq * 5 + pr) % 4)
                        # combine per tap: weight-mult + corner reduce
                        if npair == 2:
                            samp2 = mpool.tile([128, GQ * 128], BF16,
                                               tag="s2")
                        else:
                            samp2 = mpool.tile([128, GQ * 64], BF16,
                                               tag="s2s")
                        gtv = rr(gt, "p (i c j) -> p i c j", i=nslot, c=64)
                        for t in range(npair):
                            k = k0 + t
                            tt = mpool.tile([128, GQ * 256], BF16, tag="tt")
                            ttv = rr(tt, "p (g c j) -> p g c j", g=GQ, c=64)
                            nc.vector.tensor_tensor(
                                out=ttv,
                                in0=gtv[:, t * GQ:(t + 1) * GQ, :, :],
                                in1=wqv2[:, 16 * cq:16 * cq + 16, k, None,
                                         :].to_broadcast([128, GQ, 64, 4]),
                                op=ALU.mult)
                            if npair == 2:
                                s2v = rr(samp2, "p (g w2 c) -> p g w2 c",
                                         g=GQ, w2=2)
                                outv = s2v[:, :, t, :]
                            else:
                                outv = rr(samp2, "p (g c) -> p g c", g=GQ)
                            # corner sum: x-pairs first (2-wide packed),
                            # then the two contiguous jp halves.
                            u = mpool.tile([128, 2 * GQ * 64], BF16,
                                           tag="u")
                            nc.vector.tensor_tensor(
                                out=rr(u, "p (jp g c) -> p g c jp", jp=2,
                                       g=GQ),
                                in0=ttv[:, :, :, 0:2],
                                in1=ttv[:, :, :, 2:4], op=ALU.add)
                            nc.vector.tensor_tensor(
                                out=outv,
                                in0=rr(u, "p (jp gc) -> p jp gc",
                                       jp=2)[:, 0, :]
                                .rearrange("p (g c) -> p g c", g=GQ),
                                in1=rr(u, "p (jp gc) -> p jp gc",
                                       jp=2)[:, 1, :]
                                .rearrange("p (g c) -> p g c", g=GQ),
                                op=ALU.add)
                        # transpose to channel-on-partition
                        cw = 128 if npair == 2 else 64
                        sampT = mpool.tile([cw, GQ * 128], BF16, tag=f"sT{cw}")
                        for half in range(2):
                            px = ps_tx.tile([128, 1024], BF16, tag="px")
                            for j8 in range(8):
                                g16 = half * 8 + j8
                                nc.tensor.transpose(
                                    px[0:cw, j8 * 128:(j8 + 1) * 128],
                                    samp2[:, g16 * cw:(g16 + 1) * cw],
                                    identb[:, :])
                            nc.scalar.copy(
                                sampT[:, half * 1024:(half + 1) * 1024],
                                px[0:cw, :])
                        lhsT = wm2[:, pr * 64:(pr + 1) * 64] if npair == 2 \
                            else wms[:, :]
                        for gb4 in range(4):
                            nc.tensor.matmul(
                                ops[:, gb4 * 512:(gb4 + 1) * 512],
                                lhsT=lhsT,
                                rhs=sampT[:, gb4 * 512:(gb4 + 1) * 512],
                                start=(pr == 0), stop=(pr == 4))
                    nc.scalar.activation(
                        out_sb[:, cq * 2048:(cq + 1) * 2048], ops[:],
                        ACTF.Identity, bias=biast[:, 0:1])
            nc.sync.dma_start(d_out[:], out_sb[:])
    nc.compile()
    return nc


def _prep_core(inputs, core):
    b, r = core // 2, core % 2
    r0 = r * R
    keyt = np.ascontiguousarray(inputs["input_keyt"][b], np.float32)
    inter = np.ascontiguousarray(inputs["inter"][b], np.float32)
    weight = np.asarray(inputs["weight"], np.float32)
    bias = np.asarray(inputs["bias"], np.float32)
    w_om = np.asarray(inputs["w_om"], np.float32)
    b_om = np.asarray(inputs["b_om"], np.float32)

    x2full = np.concatenate([keyt, inter], axis=0)          # (128, 128, 128)
    x2c = np.zeros((128, 66, PW), np.float32)
    lo, hi = max(0, r0 - 1), min(H, r0 + R + 1)
    x2c[:, lo - (r0 - 1):hi - (r0 - 1), 1:129] = x2full[:, lo:hi, :]
    x2 = x2c.reshape(128, -1).astype(BF)

    # 2x2 patch tokens, anchors (y0, x0) in [-1, 128]^2, token [c, (jy, jx)]
    im = keyt.transpose(1, 2, 0)                            # (H, W, C)
    Z = np.zeros((H + 4, W + 4, C), np.float32)
    Z[2:H + 2, 2:W + 2] = im
    # anchor a=y0+1 in [0,130): rows y0+jy = a-1+jy -> Z[a+1+jy]
    P00 = Z[1:1 + PW, 1:1 + PW]
    P01 = Z[1:1 + PW, 2:2 + PW]
    P10 = Z[2:2 + PW, 1:1 + PW]
    P11 = Z[2:2 + PW, 2:2 + PW]
    patch = np.stack([P00, P01, P10, P11], axis=3)          # (130,130,C,4)
    patch = patch.reshape(PW * PW, 4 * C).astype(BF)

    ky = (np.arange(K2) // 3).astype(np.float32)
    kx = (np.arange(K2) % 3).astype(np.float32)
    p_ = np.arange(128, dtype=np.float32)
    g_ = np.arange(64, dtype=np.float32)
    base = np.zeros((128, 64, 18), np.float32)
    base[:, :, 0:9] = (r0 + g_[None, :, None]) - 1 + ky[None, None, :] + SH
    base[:, :, 9:18] = p_[:, None, None] - 1 + kx[None, None, :] + SH

    j_ = np.arange(128)[:, None, None]
    q_ = np.arange(16)[None, :, None]
    c_ = np.arange(4)[None, None, :]
    pg = 16 * (128 * c_ + j_) + q_                          # (128,16,4)
    hl, wl = pg // 128, pg % 128
    basew = np.zeros((128, 16, 4, 18), np.float32)
    basew[:, :, :, 0:9] = (r0 + hl)[..., None] - 1 + ky + SH
    basew[:, :, :, 9:18] = wl[..., None] - 1 + kx + SH

    womt81 = np.zeros((128, 3, 3, 32), np.float32)
    for dy in range(3):
        for dx in range(3):
            womt81[:, dy, dx, 0:27] = w_om[:, :, dy, dx].T
    W9 = weight.reshape(O, C, K2)
    wm2 = np.zeros((128, 4, 64), np.float32)
    for pr in range(4):
        for i in range(2):
            wm2[64 * i:64 * (i + 1), pr, :] = W9[:, :, 2 * pr + i].T
    wms = np.ascontiguousarray(W9[:, :, 8].T)

    return {
        "x2": x2,
        "patch": patch,
        "base": base.reshape(128, -1).astype(BF),
        "basew": basew.reshape(128, -1).astype(BF),
        "womt": womt81.reshape(128, -1).astype(BF),
        "bomt": b_om.reshape(27, 1).astype(np.float32),
        "wm2": wm2.reshape(128, -1).astype(BF),
        "wms": wms.astype(BF),
        "biast": bias.reshape(64, 1).astype(np.float32),
    }


_PROG = None


def kernel(**inputs) -> np.ndarray:
    global _PROG
    from concourse.bass_utils import run_bass_kernel_spmd
    if _PROG is None:
        _PROG = build_program()
    in_maps = [_prep_core(inputs, i) for i in range(NCORES)]
    res = run_bass_kernel_spmd(_PROG, in_maps, core_ids=list(range(NCORES)))
    out = np.zeros((B, O, H, W), np.float32)
    for i in range(NCORES):
        b, r = i // 2, i % 2
        out[b][:, r * R:(r + 1) * R, :] = res.results[i]["out"].reshape(O, R, W)
    return out


# revision 23
# speedup vs baseline: 1.4496x; 1.4496x over previous
"""Modulated deformable conv (DCNv2) Bass kernel for Trainium2, 8 NeuronCores.

Sharding: data-parallel over batch x row-halves; core i handles sample i//2,
output rows 64*(i%2) .. 64*(i%2)+63. No cross-core communication.

v3: multi-queue SWDGE gathers (4 Q7 pairs in parallel), dx-packed offset
conv (81-row lhsT), xbar DMA transposes for both om layouts, and a
2-op-per-tap combine (weight-mult + corner reduce) on a [c, corner] token.

Per-core pipeline:
  B. PE: offset/mask conv (128ch -> 27ch, 3x3) as 3 dy-packed matmuls
     (81-row lhsT = 3 dx shifts), PSUM acc; DVE sums the column-shifted
     dx blocks + bias -> om (bf16, 32 rows).
  C. Xbar DMA transpose: om -> omT natural (position-on-partition) and
     omTw wrapped (gather token order), both [*, 32]-strided bf16.
  D. DVE: fractional bilinear weights with mask + out-of-bounds validity
     folded into 4 corner weights wq[p, g, k, j] (bf16), plus patch-anchor
     gather indices (int32 -> wrapped int16).
  F. Indirect DMA gather (SWDGE): 512B tokens = 64ch x 2x2 pixel patch
     ([c, corner] layout) from a host-prebuilt patch table in HBM; one
     call per (quarter, tap-pair) = 20 calls spread over 4 SWDGE queues
     (4 Q7 descriptor-generator pairs run concurrently).
  G. DVE: per-tap corner-weight multiply (1 op) + corner reduce (1 op).
  H. PE: paired-tap transposes ([128,128] bf16) + main conv with 128-deep
     contraction (2 taps x 64ch); bias added during PSUM->SBUF copy (ACT).
"""
import sys

for _p in ("/opt/trn_rl_repo", "/root/.axon_site/_ro/trn_rl_repo"):
    if _p not in sys.path:
        sys.path.append(_p)

import numpy as np
import ml_dtypes

import concourse.bacc as bacc
import concourse.bass as bass
import concourse.mybir as mybir
import concourse.tile as tile
from concourse.masks import make_identity

F32 = mybir.dt.float32
BF16 = mybir.dt.bfloat16
I32 = mybir.dt.int32
ALU = mybir.AluOpType
ACTF = mybir.ActivationFunctionType
AXL = mybir.AxisListType
BF = ml_dtypes.bfloat16

B, C, H, W = 4, 64, 128, 128
O, K2 = 64, 9
NCORES = 8
R = H // 2             # output rows per core
PW = 130               # patch-table width (anchors -1..128)
NQ = 4                 # quarters of the per-core position space
GQ = 16                # row-chunks (=output rows) per quarter
SH = 4.0               # +4 domain shift so floor domain is positive
KOFF = -(3.0 * PW + 3.0)   # anchor idx = (y0s-3)*130 + (x0s-3)


DEBUG = False


def rr(t, spec, **kw):
    return t[:].rearrange(spec, **kw)


def build_program():
    nc = bacc.Bacc("TRN2", num_swdge_queues=4)
    d_x2 = nc.dram_tensor("x2", [128, 66 * PW], BF16, kind="ExternalInput")
    d_patch = nc.dram_tensor("patch", [PW * PW, 256], BF16, kind="ExternalInput")
    d_base = nc.dram_tensor("base", [128, 64 * 18], BF16, kind="ExternalInput")
    d_basew = nc.dram_tensor("basew", [128, 16 * 4 * 18], BF16, kind="ExternalInput")
    d_womt = nc.dram_tensor("womt", [128, 3 * 96], BF16, kind="ExternalInput")
    d_bomt = nc.dram_tensor("bomt", [27, 1], F32, kind="ExternalInput")
    d_wm2 = nc.dram_tensor("wm2", [128, 4 * 64], BF16, kind="ExternalInput")
    d_wms = nc.dram_tensor("wms", [64, 64], BF16, kind="ExternalInput")
    d_biast = nc.dram_tensor("biast", [64, 1], F32, kind="ExternalInput")
    d_out = nc.dram_tensor("out", [64, R * W], F32, kind="ExternalOutput")
    if DEBUG:
        d_dbg_om = nc.dram_tensor("dbg_om", [32, R * W], BF16,
                                  kind="ExternalOutput")
        d_dbg_omT = nc.dram_tensor("dbg_omT", [128, 64 * 32], BF16,
                                   kind="ExternalOutput")
        d_dbg_omTw = nc.dram_tensor("dbg_omTw", [128, 16 * 4 * 32], BF16,
                                    kind="ExternalOutput")
        d_dbg_wq = nc.dram_tensor("dbg_wq", [128, 64 * 9 * 4], BF16,
                                  kind="ExternalOutput")
        d_dbg_idxw = nc.dram_tensor("dbg_idxw", [128, 9 * 4 * 128],
                                    mybir.dt.int16, kind="ExternalOutput")

    with tile.TileContext(nc) as tc:
        with (
            tc.tile_pool(name="consts", bufs=1) as cpool,
        ):
            identb = cpool.tile([128, 128], BF16)
            make_identity(nc, identb[:])
            identf = cpool.tile([128, 128], F32)
            make_identity(nc, identf[:])
            womt = cpool.tile([128, 3 * 96], BF16)
            bomt = cpool.tile([27, 1], F32)
            base = cpool.tile([128, 64 * 18], BF16)
            basew = cpool.tile([128, 16 * 4 * 18], BF16)
            wm2 = cpool.tile([128, 4 * 64], BF16)
            wms = cpool.tile([64, 64], BF16)
            biast = cpool.tile([64, 1], F32)
            for sb, dr in ((womt, d_womt), (bomt, d_bomt), (base, d_base),
                           (basew, d_basew), (wm2, d_wm2), (wms, d_wms),
                           (biast, d_biast)):
                nc.sync.dma_start(sb[:], dr[:])

            # corner weights wq[p, g64, k9, j4] bf16 + gather indices
            wq = cpool.tile([128, 64 * 9 * 4], BF16)
            idxw = cpool.tile([128, 9 * 4 * 128], mybir.dt.int16)

            with (
                tc.tile_pool(name="mid", bufs=1) as midpool,
                tc.tile_pool(name="tmp", bufs=1) as tpool,
                tc.tile_pool(name="pso", bufs=2, space="PSUM") as ps_om,
                tc.tile_pool(name="pstp", bufs=2, space="PSUM") as ps_tp,
            ):
                omT = midpool.tile([128, 64 * 32], BF16)
                om = midpool.tile([32, R * W], BF16)
                om2 = midpool.tile([32, R * W], BF16)
                omTw = midpool.tile([128, 16 * 4 * 32], BF16)
                omTv = rr(omT, "p (g s) -> p g s", s=32)
                # ---- B: om conv (3 dy-packed matmuls, dx folded on DVE) ----
                x2 = midpool.tile([128, 66 * PW], BF16)
                nc.sync.dma_start(x2[:], d_x2[:])
                x2v = rr(x2, "p (r c) -> p r c", c=PW)
                omv = rr(om, "s (nt c) -> s nt c", c=512)
                om2w = rr(om2, "s (q x) -> s q x", q=16)
                for nt in range(16):
                    ps = ps_om.tile([96, 1024], F32)
                    psf = rr(ps, "p (r x) -> p r x", x=256)
                    for dy in range(3):
                        lhsT = womt[:, dy * 96:(dy + 1) * 96]
                        for r4 in range(4):
                            # start=True clears has_written for the WHOLE
                            # bank: only the first region per bank sets it.
                            nc.tensor.matmul(
                                ps[:, 256 * r4:256 * r4 + 130],
                                lhsT=lhsT,
                                rhs=x2v[:, nt * 4 + dy + r4, :],
                                start=(dy == 0 and r4 % 2 == 0),
                                stop=(dy == 2))
                    t0 = tpool.tile([27, 512], F32, tag="om_t0")
                    t1 = tpool.tile([27, 512], F32, tag="om_t1")
                    nc.vector.tensor_tensor(
                        out=rr(t0, "p (r x) -> p r x", x=128),
                        in0=psf[0:27, :, 0:128],
                        in1=bomt[:, 0:1].to_broadcast([27, 4, 128]), op=ALU.add)
                    nc.vector.tensor_tensor(
                        out=rr(t1, "p (r x) -> p r x", x=128),
                        in0=psf[32:59, :, 1:129],
                        in1=rr(t0, "p (r x) -> p r x", x=128), op=ALU.add)
                    nc.vector.tensor_tensor(
                        out=omv[0:27, nt, :].rearrange("s (r x) -> s r x",
                                                       x=128),
                        in0=psf[64:91, :, 2:130],
                        in1=rr(t1, "p (r x) -> p r x", x=128), op=ALU.add)
                    # wrap-ordered duplicate: om2[s, 512q + xw] = om[s, 16xw + q]
                    nc.vector.tensor_tensor(
                        out=om2w[0:27, :, 32 * nt:32 * nt + 32]
                        .rearrange("s q (r xh) -> s r xh q", xh=8),
                        in0=psf[64:91, :, 2:130]
                        .rearrange("s r (xh q) -> s r xh q", q=16),
                        in1=rr(t1, "p (r xh q) -> p r xh q", xh=8, q=16),
                        op=ALU.add)

                # ---- C: omT natural + omTw wrapped via xbar DMA transpose ----
                nc.sync.dma_start_transpose(omTv, om[:])
                omTwv = rr(omTw, "p (q c s) -> p q c s", q=16, c=4)
                nc.sync.dma_start_transpose(
                    rr(omTw, "p (t s) -> p t s", s=32), om2[:])
                nc.scalar.activation(omTv[:, :, 18:27], omTv[:, :, 18:27],
                                     ACTF.Sigmoid)

                # ---- D: bilinear corner weights + gather indices ----
                basev = rr(base, "p (g s) -> p g s", s=18)

                def v18(t):
                    return rr(t, "p (g s) -> p g s", s=18)

                def v9(t):
                    return rr(t, "p (g s) -> p g s", s=9)

                pypx = tpool.tile([128, 64 * 18], F32)
                nc.vector.tensor_add(out=v18(pypx), in0=omTv[:, :, 0:18],
                                     in1=basev)
                ri32 = tpool.tile([128, 64 * 18], mybir.dt.int32)
                nc.vector.tensor_copy(ri32[:], pypx[:])
                rf32 = tpool.tile([128, 64 * 18], F32)
                nc.vector.tensor_copy(rf32[:], ri32[:])
                gt_ = tpool.tile([128, 64 * 18], F32, tag="ri32")
                nc.vector.tensor_tensor(out=gt_[:], in0=rf32[:], in1=pypx[:],
                                        op=ALU.is_gt)
                flor = tpool.tile([128, 64 * 18], F32)
                nc.vector.tensor_tensor(out=flor[:], in0=rf32[:], in1=gt_[:],
                                        op=ALU.subtract)
                frac = tpool.tile([128, 64 * 18], F32)
                nc.vector.tensor_tensor(out=frac[:], in0=pypx[:], in1=flor[:],
                                        op=ALU.subtract)
                f0c = tpool.tile([128, 64 * 18], F32)
                nc.vector.tensor_scalar(f0c[:], flor[:], SH, 127.0 + SH,
                                        ALU.max, ALU.min)
                v0 = tpool.tile([128, 64 * 18], F32)
                nc.vector.tensor_tensor(out=v0[:], in0=f0c[:], in1=flor[:],
                                        op=ALU.is_equal)
                f1 = tpool.tile([128, 64 * 18], F32)
                nc.vector.tensor_scalar(f1[:], flor[:], 1.0, None, ALU.add)
                f1c = tpool.tile([128, 64 * 18], F32)
                nc.vector.tensor_scalar(f1c[:], f1[:], SH, 127.0 + SH,
                                        ALU.max, ALU.min)
                v1 = tpool.tile([128, 64 * 18], F32)
                nc.vector.tensor_tensor(out=v1[:], in0=f1c[:], in1=f1[:],
                                        op=ALU.is_equal)

                wy, wx = v18(frac)[:, :, 0:9], v18(frac)[:, :, 9:18]
                vy0, vx0 = v18(v0)[:, :, 0:9], v18(v0)[:, :, 9:18]
                vy1, vx1 = v18(v1)[:, :, 0:9], v18(v1)[:, :, 9:18]
                mskf = tpool.tile([128, 64 * 9], F32)
                nc.vector.tensor_copy(mskf[:], omTv[:, :, 18:27])
                msk = v9(mskf)

                a0 = tpool.tile([128, 64 * 9], F32)
                a1 = tpool.tile([128, 64 * 9], F32)
                b0 = tpool.tile([128, 64 * 9], F32)
                b1 = tpool.tile([128, 64 * 9], F32)
                a0v, a1v, b0v, b1v = v9(a0), v9(a1), v9(b0), v9(b1)
                nc.vector.tensor_scalar(a0[:], wy, -1.0, 1.0, ALU.mult, ALU.add)
                nc.vector.tensor_tensor(out=a0v, in0=a0v, in1=vy0, op=ALU.mult)
                nc.vector.tensor_tensor(out=a0v, in0=a0v, in1=msk, op=ALU.mult)
                nc.vector.tensor_tensor(out=a1v, in0=wy, in1=vy1, op=ALU.mult)
                nc.vector.tensor_tensor(out=a1v, in0=a1v, in1=msk, op=ALU.mult)
                nc.vector.tensor_scalar(b0[:], wx, -1.0, 1.0, ALU.mult, ALU.add)
                nc.vector.tensor_tensor(out=b0v, in0=b0v, in1=vx0, op=ALU.mult)
                nc.vector.tensor_tensor(out=b1v, in0=wx, in1=vx1, op=ALU.mult)

                wqv = rr(wq, "p (g k j) -> p g k j", g=64, k=9)
                for j, (ya, xb) in enumerate(((a0v, b0v), (a0v, b1v),
                                              (a1v, b0v), (a1v, b1v))):
                    nc.vector.tensor_tensor(
                        out=wqv[:, :, :, j], in0=ya, in1=xb, op=ALU.mult)

                # ---- D2: wrapped pipeline -> patch anchor indices ----
                def w18(t):
                    return rr(t, "p (q c s) -> p q c s", q=16, c=4, s=18)

                def w9t(t):
                    return rr(t, "p (q c s) -> p q c s", q=16, c=4, s=9)

                pypw = tpool.tile([128, 16 * 4 * 18], F32)
                nc.vector.tensor_add(out=w18(pypw),
                                     in0=omTwv[:, :, :, 0:18],
                                     in1=rr(basew, "p (q c s) -> p q c s",
                                            q=16, c=4))
                ri32w = tpool.tile([128, 16 * 4 * 18], mybir.dt.int32)
                nc.vector.tensor_copy(ri32w[:], pypw[:])
                rf32w = tpool.tile([128, 16 * 4 * 18], F32)
                nc.vector.tensor_copy(rf32w[:], ri32w[:])
                gtw_ = tpool.tile([128, 16 * 4 * 18], F32, tag="ri32w")
                nc.vector.tensor_tensor(out=gtw_[:], in0=rf32w[:], in1=pypw[:],
                                        op=ALU.is_gt)
                florw = tpool.tile([128, 16 * 4 * 18], F32)
                nc.vector.tensor_tensor(out=florw[:], in0=rf32w[:],
                                        in1=gtw_[:], op=ALU.subtract)
                # clamp anchors to [-1, 128] (+SH domain: [3, 132])
                fy0w = tpool.tile([128, 16 * 4 * 9], F32)
                nc.vector.tensor_scalar(w9t(fy0w), w18(florw)[:, :, :, 0:9],
                                        SH - 1.0, 128.0 + SH, ALU.max, ALU.min)
                gxw = tpool.tile([128, 16 * 4 * 9], F32)
                nc.vector.tensor_scalar(w9t(gxw), w18(florw)[:, :, :, 9:18],
                                        SH - 1.0, 128.0 + SH, ALU.max, ALU.min)
                idxt = tpool.tile([128, 16 * 4 * 9], F32)
                # anchor = (y0s-3)*130 + (x0s-3), y0s/x0s in +4 domain
                nc.vector.tensor_scalar(idxt[:], fy0w[:], float(PW), KOFF,
                                        ALU.mult, ALU.add)
                nc.vector.tensor_tensor(out=idxt[:], in0=idxt[:], in1=gxw[:],
                                        op=ALU.add)

                # ---- E: fold indices to wrapped int16 layout [cq][k][128]
                for k in range(K2):
                    srcT = w9t(idxt)
                    for cq in range(4):
                        pv = ps_tp.tile([16, 128], F32, tag="pidx")
                        nc.tensor.transpose(pv[:], srcT[:, :, cq, k],
                                            identf[:, :])
                        off = (cq * 9 + k) * 128
                        nc.vector.tensor_copy(idxw[0:16, off:off + 128],
                                              pv[:])
                for g in range(1, 8):
                    nc.sync.dma_start(idxw[16 * g:16 * (g + 1), :],
                                      idxw[0:16, :])
                if DEBUG:
                    nc.sync.dma_start(d_dbg_om[:], om[:])
                    nc.sync.dma_start(d_dbg_omT[:], omT[:])
                    nc.sync.dma_start(d_dbg_omTw[:], omTw[:])
                    nc.sync.dma_start(d_dbg_wq[:], wq[:])
                    nc.sync.dma_start(d_dbg_idxw[:], idxw[:])

            # ---- F/G/H: gather, combine, transpose, main conv ----
            wqv2 = rr(wq, "p (g k j) -> p g k j", g=64, k=9)
            with (
                tc.tile_pool(name="gat", bufs=4) as gpool,
                tc.tile_pool(name="gat1", bufs=2) as gpool1,
                tc.tile_pool(name="outp", bufs=1) as opool,
                tc.tile_pool(name="comb", bufs=2) as mpool,
                tc.tile_pool(name="pstx", bufs=2, space="PSUM") as ps_tx,
                tc.tile_pool(name="psmain", bufs=1, space="PSUM") as ps_main,
            ):
                out_sb = opool.tile([64, R * W], F32, tag="out_sb")
                reg2k = nc.gpsimd.to_reg(2048)
                reg4k = nc.gpsimd.to_reg(4096)
                for cq in range(NQ):
                    ops = ps_main.tile([64, 2048], F32)
                    for pr in range(5):
                        npair = 2 if pr < 4 else 1
                        nslot = GQ * npair
                        gt = (gpool if npair == 2 else gpool1).tile(
                            [128, nslot * 256], BF16, tag=f"gt{npair}")
                        k0 = 2 * pr
                        ioff = (cq * 9 + k0) * 128
                        nc.gpsimd.dma_gather(
                            rr(gt, "p (i e) -> p i e", e=256),
                            d_patch[:],
                            idxw[:, ioff:ioff + 128 * npair],
                            2048 * npair,
                            reg4k if npair == 2 else reg2k,
                            256,
                            single_packet=False,
                            queue_num=(cq * 5 + pr) % 4)
                        # combine per tap: weight-mult + corner reduce
                        if npair == 2:
                            samp2 = mpool.tile([128, GQ * 128], BF16,
                                               tag="s2")
                        else:
                            samp2 = mpool.tile([128, GQ * 64], BF16,
                                               tag="s2s")
                        gtv = rr(gt, "p (i c j) -> p i c j", i=nslot, c=64)
                        for t in range(npair):
                            k = k0 + t
                            tt = mpool.tile([128, GQ * 256], BF16, tag="tt")
                            ttv = rr(tt, "p (g c j) -> p g c j", g=GQ, c=64)
                            nc.vector.tensor_tensor(
                                out=ttv,
                                in0=gtv[:, t * GQ:(t + 1) * GQ, :, :],
                                in1=wqv2[:, 16 * cq:16 * cq + 16, k, None,
                                         :].to_broadcast([128, GQ, 64, 4]),
                                op=ALU.mult)
                            if npair == 2:
                                s2v = rr(samp2, "p (g w2 c) -> p g w2 c",
                                         g=GQ, w2=2)
                                outv = s2v[:, :, t, :]
                            else:
                                outv = rr(samp2, "p (g c) -> p g c", g=GQ)
                            with nc.allow_low_precision(
                                    reason="4-corner bilinear sum, bf16"):
                                nc.vector.tensor_reduce(
                                    out=outv, in_=ttv, axis=AXL.X,
                                    op=ALU.add)
                        # transpose to channel-on-partition
                        cw = 128 if npair == 2 else 64
                        sampT = mpool.tile([cw, GQ * 128], BF16, tag=f"sT{cw}")
                        for half in range(2):
                            px = ps_tx.tile([128, 1024], BF16, tag="px")
                            for j8 in range(8):
                                g16 = half * 8 + j8
                                nc.tensor.transpose(
                                    px[0:cw, j8 * 128:(j8 + 1) * 128],
                                    samp2[:, g16 * cw:(g16 + 1) * cw],
                                    identb[:, :])
                            nc.scalar.copy(
                                sampT[:, half * 1024:(half + 1) * 1024],
                                px[0:cw, :])
                        lhsT = wm2[:, pr * 64:(pr + 1) * 64] if npair == 2 \
                            else wms[:, :]
                        for gb4 in range(4):
                            nc.tensor.matmul(
                                ops[:, gb4 * 512:(gb4 + 1) * 512],
                                lhsT=lhsT,
                                rhs=sampT[:, gb4 * 512:(gb4 + 1) * 512],
                                start=(pr == 0), stop=(pr == 4))
                    nc.scalar.activation(
                        out_sb[:, cq * 2048:(cq + 1) * 2048], ops[:],
                        ACTF.Identity, bias=biast[:, 0:1])
            nc.sync.dma_start(d_out[:], out_sb[:])
    nc.compile()
    return nc


def _prep_core(inputs, core):
    b, r = core // 2, core % 2
    r0 = r * R
    keyt = np.ascontiguousarray(inputs["input_keyt"][b], np.float32)
    inter = np.ascontiguousarray(inputs["inter"][b], np.float32)
    weight = np.asarray(inputs["weight"], np.float32)
    bias = np.asarray(inputs["bias"], np.float32)
    w_om = np.asarray(inputs["w_om"], np.float32)
    b_om = np.asarray(inputs["b_om"], np.float32)

    x2full = np.concatenate([keyt, inter], axis=0)          # (128, 128, 128)
    x2c = np.zeros((128, 66, PW), np.float32)
    lo, hi = max(0, r0 - 1), min(H, r0 + R + 1)
    x2c[:, lo - (r0 - 1):hi - (r0 - 1), 1:129] = x2full[:, lo:hi, :]
    x2 = x2c.reshape(128, -1).astype(BF)

    # 2x2 patch tokens, anchors (y0, x0) in [-1, 128]^2, token [c, (jy, jx)]
    im = keyt.transpose(1, 2, 0)                            # (H, W, C)
    Z = np.zeros((H + 4, W + 4, C), np.float32)
    Z[2:H + 2, 2:W + 2] = im
    # anchor a=y0+1 in [0,130): rows y0+jy = a-1+jy -> Z[a+1+jy]
    P00 = Z[1:1 + PW, 1:1 + PW]
    P01 = Z[1:1 + PW, 2:2 + PW]
    P10 = Z[2:2 + PW, 1:1 + PW]
    P11 = Z[2:2 + PW, 2:2 + PW]
    patch = np.stack([P00, P01, P10, P11], axis=3)          # (130,130,C,4)
    patch = patch.reshape(PW * PW, 4 * C).astype(BF)

    ky = (np.arange(K2) // 3).astype(np.float32)
    kx = (np.arange(K2) % 3).astype(np.float32)
    p_ = np.arange(128, dtype=np.float32)
    g_ = np.arange(64, dtype=np.float32)
    base = np.zeros((128, 64, 18), np.float32)
    base[:, :, 0:9] = (r0 + g_[None, :, None]) - 1 + ky[None, None, :] + SH
    base[:, :, 9:18] = p_[:, None, None] - 1 + kx[None, None, :] + SH

    j_ = np.arange(128)[:, None, None]
    q_ = np.arange(16)[None, :, None]
    c_ = np.arange(4)[None, None, :]
    pg = 16 * (128 * c_ + j_) + q_                          # (128,16,4)
    hl, wl = pg // 128, pg % 128
    basew = np.zeros((128, 16, 4, 18), np.float32)
    basew[:, :, :, 0:9] = (r0 + hl)[..., None] - 1 + ky + SH
    basew[:, :, :, 9:18] = wl[..., None] - 1 + kx + SH

    womt81 = np.zeros((128, 3, 3, 32), np.float32)
    for dy in range(3):
        for dx in range(3):
            womt81[:, dy, dx, 0:27] = w_om[:, :, dy, dx].T
    W9 = weight.reshape(O, C, K2)
    wm2 = np.zeros((128, 4, 64), np.float32)
    for pr in range(4):
        for i in range(2):
            wm2[64 * i:64 * (i + 1), pr, :] = W9[:, :, 2 * pr + i].T
    wms = np.ascontiguousarray(W9[:, :, 8].T)

    return {
        "x2": x2,
        "patch": patch,
        "base": base.reshape(128, -1).astype(BF),
        "basew": basew.reshape(128, -1).astype(BF),
        "womt": womt81.reshape(128, -1).astype(BF),
        "bomt": b_om.reshape(27, 1).astype(np.float32),
        "wm2": wm2.reshape(128, -1).astype(BF),
        "wms": wms.astype(BF),
        "biast": bias.reshape(64, 1).astype(np.float32),
    }


_PROG = None


def kernel(**inputs) -> np.ndarray:
    global _PROG
    from concourse.bass_utils import run_bass_kernel_spmd
    if _PROG is None:
        _PROG = build_program()
    in_maps = [_prep_core(inputs, i) for i in range(NCORES)]
    res = run_bass_kernel_spmd(_PROG, in_maps, core_ids=list(range(NCORES)))
    out = np.zeros((B, O, H, W), np.float32)
    for i in range(NCORES):
        b, r = i // 2, i % 2
        out[b][:, r * R:(r + 1) * R, :] = res.results[i]["out"].reshape(O, R, W)
    return out


# revision 24
# speedup vs baseline: 1.4833x; 1.0233x over previous
"""Modulated deformable conv (DCNv2) Bass kernel for Trainium2, 8 NeuronCores.

Sharding: data-parallel over batch x row-halves; core i handles sample i//2,
output rows 64*(i%2) .. 64*(i%2)+63. No cross-core communication.

v3: multi-queue SWDGE gathers (4 Q7 pairs in parallel), dx-packed offset
conv (81-row lhsT), xbar DMA transposes for both om layouts, and a
2-op-per-tap combine (weight-mult + corner reduce) on a [c, corner] token.

Per-core pipeline:
  B. PE: offset/mask conv (128ch -> 27ch, 3x3) as 3 dy-packed matmuls
     (81-row lhsT = 3 dx shifts), PSUM acc; DVE sums the column-shifted
     dx blocks + bias -> om (bf16, 32 rows).
  C. Xbar DMA transpose: om -> omT natural (position-on-partition) and
     omTw wrapped (gather token order), both [*, 32]-strided bf16.
  D. DVE: fractional bilinear weights with mask + out-of-bounds validity
     folded into 4 corner weights wq[p, g, k, j] (bf16), plus patch-anchor
     gather indices (int32 -> wrapped int16).
  F. Indirect DMA gather (SWDGE): 512B tokens = 64ch x 2x2 pixel patch
     ([c, corner] layout) from a host-prebuilt patch table in HBM; one
     call per (quarter, tap-pair) = 20 calls spread over 4 SWDGE queues
     (4 Q7 descriptor-generator pairs run concurrently).
  G. DVE: per-tap corner-weight multiply (1 op) + corner reduce (1 op).
  H. PE: paired-tap transposes ([128,128] bf16) + main conv with 128-deep
     contraction (2 taps x 64ch); bias added during PSUM->SBUF copy (ACT).
"""
import sys

for _p in ("/opt/trn_rl_repo", "/root/.axon_site/_ro/trn_rl_repo"):
    if _p not in sys.path:
        sys.path.append(_p)

import numpy as np
import ml_dtypes

import concourse.bacc as bacc
import concourse.bass as bass
import concourse.mybir as mybir
import concourse.tile as tile
from concourse.masks import make_identity

F32 = mybir.dt.float32
BF16 = mybir.dt.bfloat16
I32 = mybir.dt.int32
ALU = mybir.AluOpType
ACTF = mybir.ActivationFunctionType
AXL = mybir.AxisListType
BF = ml_dtypes.bfloat16

B, C, H, W = 4, 64, 128, 128
O, K2 = 64, 9
NCORES = 8
R = H // 2             # output rows per core
PW = 130               # patch-table width (anchors -1..128)
NQ = 4                 # quarters of the per-core position space
GQ = 16                # row-chunks (=output rows) per quarter
SH = 4.0               # +4 domain shift so floor domain is positive
KOFF = -(3.0 * PW + 3.0)   # anchor idx = (y0s-3)*130 + (x0s-3)


DEBUG = False


def rr(t, spec, **kw):
    return t[:].rearrange(spec, **kw)


def build_program():
    nc = bacc.Bacc("TRN2", num_swdge_queues=4)
    d_x2 = nc.dram_tensor("x2", [128, 66 * PW], BF16, kind="ExternalInput")
    d_patch = nc.dram_tensor("patch", [PW * PW, 256], BF16, kind="ExternalInput")
    d_base = nc.dram_tensor("base", [128, 64 * 18], BF16, kind="ExternalInput")
    d_basew = nc.dram_tensor("basew", [128, 16 * 4 * 18], BF16, kind="ExternalInput")
    d_womt = nc.dram_tensor("womt", [128, 3 * 96], BF16, kind="ExternalInput")
    d_bomt = nc.dram_tensor("bomt", [27, 1], F32, kind="ExternalInput")
    d_wm2 = nc.dram_tensor("wm2", [128, 4 * 64], BF16, kind="ExternalInput")
    d_wms = nc.dram_tensor("wms", [64, 64], BF16, kind="ExternalInput")
    d_biast = nc.dram_tensor("biast", [64, 1], F32, kind="ExternalInput")
    d_out = nc.dram_tensor("out", [64, R * W], F32, kind="ExternalOutput")
    if DEBUG:
        d_dbg_om = nc.dram_tensor("dbg_om", [32, R * W], BF16,
                                  kind="ExternalOutput")
        d_dbg_omT = nc.dram_tensor("dbg_omT", [128, 64 * 32], BF16,
                                   kind="ExternalOutput")
        d_dbg_omTw = nc.dram_tensor("dbg_omTw", [128, 16 * 4 * 32], BF16,
                                    kind="ExternalOutput")
        d_dbg_wq = nc.dram_tensor("dbg_wq", [128, 64 * 9 * 4], BF16,
                                  kind="ExternalOutput")
        d_dbg_idxw = nc.dram_tensor("dbg_idxw", [128, 9 * 4 * 128],
                                    mybir.dt.int16, kind="ExternalOutput")

    with tile.TileContext(nc) as tc:
        with (
            tc.tile_pool(name="consts", bufs=1) as cpool,
        ):
            identb = cpool.tile([128, 128], BF16)
            make_identity(nc, identb[:])
            identf = cpool.tile([128, 128], F32)
            make_identity(nc, identf[:])
            womt = cpool.tile([128, 3 * 96], BF16)
            bomt = cpool.tile([27, 1], F32)
            base = cpool.tile([128, 64 * 18], BF16)
            basew = cpool.tile([128, 16 * 4 * 18], BF16)
            wm2 = cpool.tile([128, 4 * 64], BF16)
            wms = cpool.tile([64, 64], BF16)
            biast = cpool.tile([64, 1], F32)
            for sb, dr in ((womt, d_womt), (bomt, d_bomt), (base, d_base),
                           (basew, d_basew), (wm2, d_wm2), (wms, d_wms),
                           (biast, d_biast)):
                nc.sync.dma_start(sb[:], dr[:])

            # corner weights wq[p, g64, k9, j4] bf16 + gather indices
            wq = cpool.tile([128, 64 * 9 * 4], BF16)
            idxw = cpool.tile([128, 9 * 4 * 128], mybir.dt.int16)

            with (
                tc.tile_pool(name="mid", bufs=1) as midpool,
                tc.tile_pool(name="tmp", bufs=1) as tpool,
                tc.tile_pool(name="pso", bufs=2, space="PSUM") as ps_om,
                tc.tile_pool(name="pstp", bufs=2, space="PSUM") as ps_tp,
            ):
                omT = midpool.tile([128, 64 * 32], BF16)
                om = midpool.tile([32, R * W], BF16)
                om2 = midpool.tile([32, R * W], BF16)
                omTw = midpool.tile([128, 16 * 4 * 32], BF16)
                omTv = rr(omT, "p (g s) -> p g s", s=32)
                # ---- B: om conv (3 dy-packed matmuls, dx folded on DVE) ----
                x2 = midpool.tile([128, 66 * PW], BF16)
                nc.sync.dma_start(x2[:], d_x2[:])
                x2v = rr(x2, "p (r c) -> p r c", c=PW)
                omv = rr(om, "s (nt c) -> s nt c", c=512)
                om2w = rr(om2, "s (q x) -> s q x", q=16)
                for nt in range(16):
                    ps = ps_om.tile([96, 1024], F32)
                    psf = rr(ps, "p (r x) -> p r x", x=256)
                    for dy in range(3):
                        lhsT = womt[:, dy * 96:(dy + 1) * 96]
                        for r4 in range(4):
                            # start=True clears has_written for the WHOLE
                            # bank: only the first region per bank sets it.
                            nc.tensor.matmul(
                                ps[:, 256 * r4:256 * r4 + 130],
                                lhsT=lhsT,
                                rhs=x2v[:, nt * 4 + dy + r4, :],
                                start=(dy == 0 and r4 % 2 == 0),
                                stop=(dy == 2))
                    t0 = tpool.tile([27, 512], F32, tag="om_t0")
                    t1 = tpool.tile([27, 512], F32, tag="om_t1")
                    nc.vector.tensor_tensor(
                        out=rr(t0, "p (r x) -> p r x", x=128),
                        in0=psf[0:27, :, 0:128],
                        in1=bomt[:, 0:1].to_broadcast([27, 4, 128]), op=ALU.add)
                    nc.vector.tensor_tensor(
                        out=rr(t1, "p (r x) -> p r x", x=128),
                        in0=psf[32:59, :, 1:129],
                        in1=rr(t0, "p (r x) -> p r x", x=128), op=ALU.add)
                    nc.vector.tensor_tensor(
                        out=omv[0:27, nt, :].rearrange("s (r x) -> s r x",
                                                       x=128),
                        in0=psf[64:91, :, 2:130],
                        in1=rr(t1, "p (r x) -> p r x", x=128), op=ALU.add)
                    # wrap-ordered duplicate: om2[s, 512q + xw] = om[s, 16xw + q]
                    nc.vector.tensor_tensor(
                        out=om2w[0:27, :, 32 * nt:32 * nt + 32]
                        .rearrange("s q (r xh) -> s r xh q", xh=8),
                        in0=psf[64:91, :, 2:130]
                        .rearrange("s r (xh q) -> s r xh q", q=16),
                        in1=rr(t1, "p (r xh q) -> p r xh q", xh=8, q=16),
                        op=ALU.add)

                # ---- C: omT natural + omTw wrapped via xbar DMA transpose ----
                nc.sync.dma_start_transpose(omTv, om[:])
                omTwv = rr(omTw, "p (q c s) -> p q c s", q=16, c=4)
                nc.sync.dma_start_transpose(
                    rr(omTw, "p (t s) -> p t s", s=32), om2[:])
                nc.scalar.activation(omTv[:, :, 18:27], omTv[:, :, 18:27],
                                     ACTF.Sigmoid)

                # ---- D2: wrapped pipeline -> patch anchor indices ----
                def w18(t):
                    return rr(t, "p (q c s) -> p q c s", q=16, c=4, s=18)

                def w9t(t):
                    return rr(t, "p (q c s) -> p q c s", q=16, c=4, s=9)

                pypw = tpool.tile([128, 16 * 4 * 18], F32)
                nc.vector.tensor_add(out=w18(pypw),
                                     in0=omTwv[:, :, :, 0:18],
                                     in1=rr(basew, "p (q c s) -> p q c s",
                                            q=16, c=4))
                ri32w = tpool.tile([128, 16 * 4 * 18], mybir.dt.int32)
                nc.vector.tensor_copy(ri32w[:], pypw[:])
                rf32w = tpool.tile([128, 16 * 4 * 18], F32)
                nc.vector.tensor_copy(rf32w[:], ri32w[:])
                gtw_ = tpool.tile([128, 16 * 4 * 18], F32, tag="ri32w")
                nc.vector.tensor_tensor(out=gtw_[:], in0=rf32w[:], in1=pypw[:],
                                        op=ALU.is_gt)
                florw = tpool.tile([128, 16 * 4 * 18], F32)
                nc.vector.tensor_tensor(out=florw[:], in0=rf32w[:],
                                        in1=gtw_[:], op=ALU.subtract)
                # clamp anchors to [-1, 128] (+SH domain: [3, 132])
                fy0w = tpool.tile([128, 16 * 4 * 9], F32)
                nc.vector.tensor_scalar(w9t(fy0w), w18(florw)[:, :, :, 0:9],
                                        SH - 1.0, 128.0 + SH, ALU.max, ALU.min)
                gxw = tpool.tile([128, 16 * 4 * 9], F32)
                nc.vector.tensor_scalar(w9t(gxw), w18(florw)[:, :, :, 9:18],
                                        SH - 1.0, 128.0 + SH, ALU.max, ALU.min)
                idxt = tpool.tile([128, 16 * 4 * 9], F32)
                # anchor = (y0s-3)*130 + (x0s-3), y0s/x0s in +4 domain
                nc.vector.tensor_scalar(idxt[:], fy0w[:], float(PW), KOFF,
                                        ALU.mult, ALU.add)
                nc.vector.tensor_tensor(out=idxt[:], in0=idxt[:], in1=gxw[:],
                                        op=ALU.add)

                # ---- E: fold indices to wrapped int16 layout [cq][k][128]
                for k in range(K2):
                    srcT = w9t(idxt)
                    for cq in range(4):
                        pv = ps_tp.tile([16, 128], F32, tag="pidx")
                        nc.tensor.transpose(pv[:], srcT[:, :, cq, k],
                                            identf[:, :])
                        off = (cq * 9 + k) * 128
                        nc.vector.tensor_copy(idxw[0:16, off:off + 128],
                                              pv[:])
                for g in range(1, 8):
                    nc.sync.dma_start(idxw[16 * g:16 * (g + 1), :],
                                      idxw[0:16, :])
                # ---- D: bilinear corner weights + gather indices ----
                basev = rr(base, "p (g s) -> p g s", s=18)

                def v18(t):
                    return rr(t, "p (g s) -> p g s", s=18)

                def v9(t):
                    return rr(t, "p (g s) -> p g s", s=9)

                pypx = tpool.tile([128, 64 * 18], F32)
                nc.vector.tensor_add(out=v18(pypx), in0=omTv[:, :, 0:18],
                                     in1=basev)
                ri32 = tpool.tile([128, 64 * 18], mybir.dt.int32)
                nc.vector.tensor_copy(ri32[:], pypx[:])
                rf32 = tpool.tile([128, 64 * 18], F32)
                nc.vector.tensor_copy(rf32[:], ri32[:])
                gt_ = tpool.tile([128, 64 * 18], F32, tag="ri32")
                nc.vector.tensor_tensor(out=gt_[:], in0=rf32[:], in1=pypx[:],
                                        op=ALU.is_gt)
                flor = tpool.tile([128, 64 * 18], F32)
                nc.vector.tensor_tensor(out=flor[:], in0=rf32[:], in1=gt_[:],
                                        op=ALU.subtract)
                frac = tpool.tile([128, 64 * 18], F32)
                nc.vector.tensor_tensor(out=frac[:], in0=pypx[:], in1=flor[:],
                                        op=ALU.subtract)
                f0c = tpool.tile([128, 64 * 18], F32)
                nc.vector.tensor_scalar(f0c[:], flor[:], SH, 127.0 + SH,
                                        ALU.max, ALU.min)
                v0 = tpool.tile([128, 64 * 18], F32)
                nc.vector.tensor_tensor(out=v0[:], in0=f0c[:], in1=flor[:],
                                        op=ALU.is_equal)
                f1 = tpool.tile([128, 64 * 18], F32)
                nc.vector.tensor_scalar(f1[:], flor[:], 1.0, None, ALU.add)
                f1c = tpool.tile([128, 64 * 18], F32)
                nc.vector.tensor_scalar(f1c[:], f1[:], SH, 127.0 + SH,
                                        ALU.max, ALU.min)
                v1 = tpool.tile([128, 64 * 18], F32)
                nc.vector.tensor_tensor(out=v1[:], in0=f1c[:], in1=f1[:],
                                        op=ALU.is_equal)

                wy, wx = v18(frac)[:, :, 0:9], v18(frac)[:, :, 9:18]
                vy0, vx0 = v18(v0)[:, :, 0:9], v18(v0)[:, :, 9:18]
                vy1, vx1 = v18(v1)[:, :, 0:9], v18(v1)[:, :, 9:18]
                mskf = tpool.tile([128, 64 * 9], F32)
                nc.vector.tensor_copy(mskf[:], omTv[:, :, 18:27])
                msk = v9(mskf)

                a0 = tpool.tile([128, 64 * 9], F32)
                a1 = tpool.tile([128, 64 * 9], F32)
                b0 = tpool.tile([128, 64 * 9], F32)
                b1 = tpool.tile([128, 64 * 9], F32)
                a0v, a1v, b0v, b1v = v9(a0), v9(a1), v9(b0), v9(b1)
                nc.vector.tensor_scalar(a0[:], wy, -1.0, 1.0, ALU.mult, ALU.add)
                nc.vector.tensor_tensor(out=a0v, in0=a0v, in1=vy0, op=ALU.mult)
                nc.vector.tensor_tensor(out=a0v, in0=a0v, in1=msk, op=ALU.mult)
                nc.vector.tensor_tensor(out=a1v, in0=wy, in1=vy1, op=ALU.mult)
                nc.vector.tensor_tensor(out=a1v, in0=a1v, in1=msk, op=ALU.mult)
                nc.vector.tensor_scalar(b0[:], wx, -1.0, 1.0, ALU.mult, ALU.add)
                nc.vector.tensor_tensor(out=b0v, in0=b0v, in1=vx0, op=ALU.mult)
                nc.vector.tensor_tensor(out=b1v, in0=wx, in1=vx1, op=ALU.mult)

                wqv = rr(wq, "p (g k j) -> p g k j", g=64, k=9)
                for j, (ya, xb) in enumerate(((a0v, b0v), (a0v, b1v),
                                              (a1v, b0v), (a1v, b1v))):
                    nc.vector.tensor_tensor(
                        out=wqv[:, :, :, j], in0=ya, in1=xb, op=ALU.mult)

                if DEBUG:
                    nc.sync.dma_start(d_dbg_om[:], om[:])
                    nc.sync.dma_start(d_dbg_omT[:], omT[:])
                    nc.sync.dma_start(d_dbg_omTw[:], omTw[:])
                    nc.sync.dma_start(d_dbg_wq[:], wq[:])
                    nc.sync.dma_start(d_dbg_idxw[:], idxw[:])

            # ---- F/G/H: gather, combine, transpose, main conv ----
            wqv2 = rr(wq, "p (g k j) -> p g k j", g=64, k=9)
            with (
                tc.tile_pool(name="gat", bufs=4) as gpool,
                tc.tile_pool(name="gat1", bufs=2) as gpool1,
                tc.tile_pool(name="outp", bufs=1) as opool,
                tc.tile_pool(name="comb", bufs=2) as mpool,
                tc.tile_pool(name="pstx", bufs=2, space="PSUM") as ps_tx,
                tc.tile_pool(name="psmain", bufs=1, space="PSUM") as ps_main,
            ):
                out_sb = opool.tile([64, R * W], F32, tag="out_sb")
                reg2k = nc.gpsimd.to_reg(2048)
                reg4k = nc.gpsimd.to_reg(4096)
                for cq in range(NQ):
                    ops = ps_main.tile([64, 2048], F32)
                    for pr in range(5):
                        npair = 2 if pr < 4 else 1
                        nslot = GQ * npair
                        gt = (gpool if npair == 2 else gpool1).tile(
                            [128, nslot * 256], BF16, tag=f"gt{npair}")
                        k0 = 2 * pr
                        ioff = (cq * 9 + k0) * 128
                        nc.gpsimd.dma_gather(
                            rr(gt, "p (i e) -> p i e", e=256),
                            d_patch[:],
                            idxw[:, ioff:ioff + 128 * npair],
                            2048 * npair,
                            reg4k if npair == 2 else reg2k,
                            256,
                            single_packet=False,
                            queue_num=(cq * 5 + pr) % 4)
                        # combine per tap: weight-mult + corner reduce
                        if npair == 2:
                            samp2 = mpool.tile([128, GQ * 128], BF16,
                                               tag="s2")
                        else:
                            samp2 = mpool.tile([128, GQ * 64], BF16,
                                               tag="s2s")
                        gtv = rr(gt, "p (i c j) -> p i c j", i=nslot, c=64)
                        for t in range(npair):
                            k = k0 + t
                            tt = mpool.tile([128, GQ * 256], BF16, tag="tt")
                            ttv = rr(tt, "p (g c j) -> p g c j", g=GQ, c=64)
                            nc.vector.tensor_tensor(
                                out=ttv,
                                in0=gtv[:, t * GQ:(t + 1) * GQ, :, :],
                                in1=wqv2[:, 16 * cq:16 * cq + 16, k, None,
                                         :].to_broadcast([128, GQ, 64, 4]),
                                op=ALU.mult)
                            if npair == 2:
                                s2v = rr(samp2, "p (g w2 c) -> p g w2 c",
                                         g=GQ, w2=2)
                                outv = s2v[:, :, t, :]
                            else:
                                outv = rr(samp2, "p (g c) -> p g c", g=GQ)
                            with nc.allow_low_precision(
                                    reason="4-corner bilinear sum, bf16"):
                                nc.vector.tensor_reduce(
                                    out=outv, in_=ttv, axis=AXL.X,
                                    op=ALU.add)
                        # transpose to channel-on-partition
                        cw = 128 if npair == 2 else 64
                        sampT = mpool.tile([cw, GQ * 128], BF16, tag=f"sT{cw}")
                        for half in range(2):
                            px = ps_tx.tile([128, 1024], BF16, tag="px")
                            for j8 in range(8):
                                g16 = half * 8 + j8
                                nc.tensor.transpose(
                                    px[0:cw, j8 * 128:(j8 + 1) * 128],
                                    samp2[:, g16 * cw:(g16 + 1) * cw],
                                    identb[:, :])
                            nc.scalar.copy(
                                sampT[:, half * 1024:(half + 1) * 1024],
                                px[0:cw, :])
                        lhsT = wm2[:, pr * 64:(pr + 1) * 64] if npair == 2 \
                            else wms[:, :]
                        for gb4 in range(4):
                            nc.tensor.matmul(
                                ops[:, gb4 * 512:(gb4 + 1) * 512],
                                lhsT=lhsT,
                                rhs=sampT[:, gb4 * 512:(gb4 + 1) * 512],
                                start=(pr == 0), stop=(pr == 4))
                    nc.scalar.activation(
                        out_sb[:, cq * 2048:(cq + 1) * 2048], ops[:],
                        ACTF.Identity, bias=biast[:, 0:1])
            nc.sync.dma_start(d_out[:], out_sb[:])
    nc.compile()
    return nc


def _prep_core(inputs, core):
    b, r = core // 2, core % 2
    r0 = r * R
    keyt = np.ascontiguousarray(inputs["input_keyt"][b], np.float32)
    inter = np.ascontiguousarray(inputs["inter"][b], np.float32)
    weight = np.asarray(inputs["weight"], np.float32)
    bias = np.asarray(inputs["bias"], np.float32)
    w_om = np.asarray(inputs["w_om"], np.float32)
    b_om = np.asarray(inputs["b_om"], np.float32)

    x2full = np.concatenate([keyt, inter], axis=0)          # (128, 128, 128)
    x2c = np.zeros((128, 66, PW), np.float32)
    lo, hi = max(0, r0 - 1), min(H, r0 + R + 1)
    x2c[:, lo - (r0 - 1):hi - (r0 - 1), 1:129] = x2full[:, lo:hi, :]
    x2 = x2c.reshape(128, -1).astype(BF)

    # 2x2 patch tokens, anchors (y0, x0) in [-1, 128]^2, token [c, (jy, jx)]
    im = keyt.transpose(1, 2, 0)                            # (H, W, C)
    Z = np.zeros((H + 4, W + 4, C), np.float32)
    Z[2:H + 2, 2:W + 2] = im
    # anchor a=y0+1 in [0,130): rows y0+jy = a-1+jy -> Z[a+1+jy]
    P00 = Z[1:1 + PW, 1:1 + PW]
    P01 = Z[1:1 + PW, 2:2 + PW]
    P10 = Z[2:2 + PW, 1:1 + PW]
    P11 = Z[2:2 + PW, 2:2 + PW]
    patch = np.stack([P00, P01, P10, P11], axis=3)          # (130,130,C,4)
    patch = patch.reshape(PW * PW, 4 * C).astype(BF)

    ky = (np.arange(K2) // 3).astype(np.float32)
    kx = (np.arange(K2) % 3).astype(np.float32)
    p_ = np.arange(128, dtype=np.float32)
    g_ = np.arange(64, dtype=np.float32)
    base = np.zeros((128, 64, 18), np.float32)
    base[:, :, 0:9] = (r0 + g_[None, :, None]) - 1 + ky[None, None, :] + SH
    base[:, :, 9:18] = p_[:, None, None] - 1 + kx[None, None, :] + SH

    j_ = np.arange(128)[:, None, None]
    q_ = np.arange(16)[None, :, None]
    c_ = np.arange(4)[None, None, :]
    pg = 16 * (128 * c_ + j_) + q_                          # (128,16,4)
    hl, wl = pg // 128, pg % 128
    basew = np.zeros((128, 16, 4, 18), np.float32)
    basew[:, :, :, 0:9] = (r0 + hl)[..., None] - 1 + ky + SH
    basew[:, :, :, 9:18] = wl[..., None] - 1 + kx + SH

    womt81 = np.zeros((128, 3, 3, 32), np.float32)
    for dy in range(3):
        for dx in range(3):
            womt81[:, dy, dx, 0:27] = w_om[:, :, dy, dx].T
    W9 = weight.reshape(O, C, K2)
    wm2 = np.zeros((128, 4, 64), np.float32)
    for pr in range(4):
        for i in range(2):
            wm2[64 * i:64 * (i + 1), pr, :] = W9[:, :, 2 * pr + i].T
    wms = np.ascontiguousarray(W9[:, :, 8].T)

    return {
        "x2": x2,
        "patch": patch,
        "base": base.reshape(128, -1).astype(BF),
        "basew": basew.reshape(128, -1).astype(BF),
        "womt": womt81.reshape(128, -1).astype(BF),
        "bomt": b_om.reshape(27, 1).astype(np.float32),
        "wm2": wm2.reshape(128, -1).astype(BF),
        "wms": wms.astype(BF),
        "biast": bias.reshape(64, 1).astype(np.float32),
    }


_PROG = None


def kernel(**inputs) -> np.ndarray:
    global _PROG
    from concourse.bass_utils import run_bass_kernel_spmd
    if _PROG is None:
        _PROG = build_program()
    in_maps = [_prep_core(inputs, i) for i in range(NCORES)]
    res = run_bass_kernel_spmd(_PROG, in_maps, core_ids=list(range(NCORES)))
    out = np.zeros((B, O, H, W), np.float32)
    for i in range(NCORES):
        b, r = i // 2, i % 2
        out[b][:, r * R:(r + 1) * R, :] = res.results[i]["out"].reshape(O, R, W)
    return out


# revision 25
# speedup vs baseline: 1.5131x; 1.0201x over previous
"""Modulated deformable conv (DCNv2) Bass kernel for Trainium2, 8 NeuronCores.

Sharding: data-parallel over batch x row-halves; core i handles sample i//2,
output rows 64*(i%2) .. 64*(i%2)+63. No cross-core communication.

v3: multi-queue SWDGE gathers (4 Q7 pairs in parallel), dx-packed offset
conv (81-row lhsT), xbar DMA transposes for both om layouts, and a
2-op-per-tap combine (weight-mult + corner reduce) on a [c, corner] token.

Per-core pipeline:
  B. PE: offset/mask conv (128ch -> 27ch, 3x3) as 3 dy-packed matmuls
     (81-row lhsT = 3 dx shifts), PSUM acc; DVE sums the column-shifted
     dx blocks + bias -> om (bf16, 32 rows).
  C. Xbar DMA transpose: om -> omT natural (position-on-partition) and
     omTw wrapped (gather token order), both [*, 32]-strided bf16.
  D. DVE: fractional bilinear weights with mask + out-of-bounds validity
     folded into 4 corner weights wq[p, g, k, j] (bf16), plus patch-anchor
     gather indices (int32 -> wrapped int16).
  F. Indirect DMA gather (SWDGE): 512B tokens = 64ch x 2x2 pixel patch
     ([c, corner] layout) from a host-prebuilt patch table in HBM; one
     call per (quarter, tap-pair) = 20 calls spread over 4 SWDGE queues
     (4 Q7 descriptor-generator pairs run concurrently).
  G. DVE: per-tap corner-weight multiply (1 op) + corner reduce (1 op).
  H. PE: paired-tap transposes ([128,128] bf16) + main conv with 128-deep
     contraction (2 taps x 64ch); bias added during PSUM->SBUF copy (ACT).
"""
import sys

for _p in ("/opt/trn_rl_repo", "/root/.axon_site/_ro/trn_rl_repo"):
    if _p not in sys.path:
        sys.path.append(_p)

import numpy as np
import ml_dtypes

import concourse.bacc as bacc
import concourse.bass as bass
import concourse.mybir as mybir
import concourse.tile as tile
from concourse.masks import make_identity

F32 = mybir.dt.float32
BF16 = mybir.dt.bfloat16
I32 = mybir.dt.int32
ALU = mybir.AluOpType
ACTF = mybir.ActivationFunctionType
AXL = mybir.AxisListType
BF = ml_dtypes.bfloat16

B, C, H, W = 4, 64, 128, 128
O, K2 = 64, 9
NCORES = 8
R = H // 2             # output rows per core
PW = 130               # patch-table width (anchors -1..128)
NQ = 4                 # quarters of the per-core position space
GQ = 16                # row-chunks (=output rows) per quarter
SH = 4.0               # +4 domain shift so floor domain is positive
KOFF = -(3.0 * PW + 3.0)   # anchor idx = (y0s-3)*130 + (x0s-3)


DEBUG = False


def rr(t, spec, **kw):
    return t[:].rearrange(spec, **kw)


def build_program():
    nc = bacc.Bacc("TRN2", num_swdge_queues=4)
    d_x2 = nc.dram_tensor("x2", [128, 66 * PW], BF16, kind="ExternalInput")
    d_patch = nc.dram_tensor("patch", [PW * PW, 256], BF16, kind="ExternalInput")
    d_base = nc.dram_tensor("base", [128, 64 * 18], BF16, kind="ExternalInput")
    d_basew = nc.dram_tensor("basew", [128, 16 * 4 * 18], BF16, kind="ExternalInput")
    d_womt = nc.dram_tensor("womt", [128, 3 * 96], BF16, kind="ExternalInput")
    d_bomt = nc.dram_tensor("bomt", [27, 1], F32, kind="ExternalInput")
    d_wm2 = nc.dram_tensor("wm2", [128, 4 * 64], BF16, kind="ExternalInput")
    d_wms = nc.dram_tensor("wms", [64, 64], BF16, kind="ExternalInput")
    d_biast = nc.dram_tensor("biast", [64, 1], F32, kind="ExternalInput")
    d_out = nc.dram_tensor("out", [64, R * W], F32, kind="ExternalOutput")
    if DEBUG:
        d_dbg_om = nc.dram_tensor("dbg_om", [32, R * W], BF16,
                                  kind="ExternalOutput")
        d_dbg_omT = nc.dram_tensor("dbg_omT", [128, 64 * 32], BF16,
                                   kind="ExternalOutput")
        d_dbg_omTw = nc.dram_tensor("dbg_omTw", [128, 16 * 4 * 32], BF16,
                                    kind="ExternalOutput")
        d_dbg_wq = nc.dram_tensor("dbg_wq", [128, 64 * 9 * 4], BF16,
                                  kind="ExternalOutput")
        d_dbg_idxw = nc.dram_tensor("dbg_idxw", [128, 9 * 4 * 128],
                                    mybir.dt.int16, kind="ExternalOutput")

    with tile.TileContext(nc) as tc:
        with (
            tc.tile_pool(name="consts", bufs=1) as cpool,
        ):
            identb = cpool.tile([128, 128], BF16)
            make_identity(nc, identb[:])
            identf = cpool.tile([128, 128], F32)
            make_identity(nc, identf[:])
            womt = cpool.tile([128, 3 * 96], BF16)
            bomt = cpool.tile([27, 1], F32)
            base = cpool.tile([128, 64 * 18], BF16)
            basew = cpool.tile([128, 16 * 4 * 18], BF16)
            wm2 = cpool.tile([128, 4 * 64], BF16)
            wms = cpool.tile([64, 64], BF16)
            biast = cpool.tile([64, 1], F32)
            for sb, dr in ((womt, d_womt), (bomt, d_bomt), (base, d_base),
                           (basew, d_basew), (wm2, d_wm2), (wms, d_wms),
                           (biast, d_biast)):
                nc.sync.dma_start(sb[:], dr[:])

            # corner weights wq[p, g64, k9, j4] bf16 + gather indices
            wq = cpool.tile([128, 64 * 9 * 4], BF16)
            idxw = cpool.tile([128, 9 * 4 * 128], mybir.dt.int16)

            with (
                tc.tile_pool(name="mid", bufs=1) as midpool,
                tc.tile_pool(name="tmp", bufs=1) as tpool,
                tc.tile_pool(name="pso", bufs=2, space="PSUM") as ps_om,
                tc.tile_pool(name="pstp", bufs=2, space="PSUM") as ps_tp,
            ):
                omT = midpool.tile([128, 64 * 32], BF16)
                om = midpool.tile([32, R * W], BF16)
                om2 = midpool.tile([32, R * W], BF16)
                omTw = midpool.tile([128, 16 * 4 * 32], BF16)
                omTv = rr(omT, "p (g s) -> p g s", s=32)
                # ---- B: om conv (3 dy-packed matmuls, dx folded on DVE) ----
                x2 = midpool.tile([128, 66 * PW], BF16)
                nc.sync.dma_start(x2[:], d_x2[:])
                x2v = rr(x2, "p (r c) -> p r c", c=PW)
                omv = rr(om, "s (nt c) -> s nt c", c=512)
                om2w = rr(om2, "s (q x) -> s q x", q=16)
                for nt in range(16):
                    ps = ps_om.tile([96, 1024], F32)
                    psf = rr(ps, "p (r x) -> p r x", x=256)
                    for dy in range(3):
                        lhsT = womt[:, dy * 96:(dy + 1) * 96]
                        for r4 in range(4):
                            # start=True clears has_written for the WHOLE
                            # bank: only the first region per bank sets it.
                            nc.tensor.matmul(
                                ps[:, 256 * r4:256 * r4 + 130],
                                lhsT=lhsT,
                                rhs=x2v[:, nt * 4 + dy + r4, :],
                                start=(dy == 0 and r4 % 2 == 0),
                                stop=(dy == 2))
                    t0 = tpool.tile([27, 512], F32, tag="om_t0")
                    t1 = tpool.tile([27, 512], F32, tag="om_t1")
                    nc.vector.tensor_tensor(
                        out=rr(t0, "p (r x) -> p r x", x=128),
                        in0=psf[0:27, :, 0:128],
                        in1=bomt[:, 0:1].to_broadcast([27, 4, 128]), op=ALU.add)
                    nc.vector.tensor_tensor(
                        out=rr(t1, "p (r x) -> p r x", x=128),
                        in0=psf[32:59, :, 1:129],
                        in1=rr(t0, "p (r x) -> p r x", x=128), op=ALU.add)
                    nc.vector.tensor_tensor(
                        out=omv[0:27, nt, :].rearrange("s (r x) -> s r x",
                                                       x=128),
                        in0=psf[64:91, :, 2:130],
                        in1=rr(t1, "p (r x) -> p r x", x=128), op=ALU.add)
                    # wrap-ordered duplicate: om2[s, 512q + xw] = om[s, 16xw + q]
                    nc.vector.tensor_tensor(
                        out=om2w[0:27, :, 32 * nt:32 * nt + 32]
                        .rearrange("s q (r xh) -> s r xh q", xh=8),
                        in0=psf[64:91, :, 2:130]
                        .rearrange("s r (xh q) -> s r xh q", q=16),
                        in1=rr(t1, "p (r xh q) -> p r xh q", xh=8, q=16),
                        op=ALU.add)

                # ---- C: omT natural + omTw wrapped via xbar DMA transpose ----
                nc.sync.dma_start_transpose(omTv, om[:])
                omTwv = rr(omTw, "p (q c s) -> p q c s", q=16, c=4)
                nc.sync.dma_start_transpose(
                    rr(omTw, "p (t s) -> p t s", s=32), om2[:])
                nc.scalar.activation(omTv[:, :, 18:27], omTv[:, :, 18:27],
                                     ACTF.Sigmoid)

                # ---- D2: wrapped pipeline -> patch anchor indices ----
                def w18(t):
                    return rr(t, "p (q c s) -> p q c s", q=16, c=4, s=18)

                def w9t(t):
                    return rr(t, "p (q c s) -> p q c s", q=16, c=4, s=9)

                pypw = tpool.tile([128, 16 * 4 * 18], F32)
                nc.vector.tensor_add(out=w18(pypw),
                                     in0=omTwv[:, :, :, 0:18],
                                     in1=rr(basew, "p (q c s) -> p q c s",
                                            q=16, c=4))
                ri32w = tpool.tile([128, 16 * 4 * 18], mybir.dt.int32)
                nc.vector.tensor_copy(ri32w[:], pypw[:])
                rf32w = tpool.tile([128, 16 * 4 * 18], F32)
                nc.vector.tensor_copy(rf32w[:], ri32w[:])
                gtw_ = tpool.tile([128, 16 * 4 * 18], F32, tag="ri32w")
                nc.vector.tensor_tensor(out=gtw_[:], in0=rf32w[:], in1=pypw[:],
                                        op=ALU.is_gt)
                florw = tpool.tile([128, 16 * 4 * 18], F32)
                nc.vector.tensor_tensor(out=florw[:], in0=rf32w[:],
                                        in1=gtw_[:], op=ALU.subtract)
                # clamp anchors to [-1, 128] (+SH domain: [3, 132])
                fy0w = tpool.tile([128, 16 * 4 * 9], F32)
                nc.vector.tensor_scalar(w9t(fy0w), w18(florw)[:, :, :, 0:9],
                                        SH - 1.0, 128.0 + SH, ALU.max, ALU.min)
                gxw = tpool.tile([128, 16 * 4 * 9], F32)
                nc.vector.tensor_scalar(w9t(gxw), w18(florw)[:, :, :, 9:18],
                                        SH - 1.0, 128.0 + SH, ALU.max, ALU.min)
                idxt = tpool.tile([128, 16 * 4 * 9], F32)
                # anchor = (y0s-3)*130 + (x0s-3), y0s/x0s in +4 domain
                nc.vector.tensor_scalar(idxt[:], fy0w[:], float(PW), KOFF,
                                        ALU.mult, ALU.add)
                nc.vector.tensor_tensor(out=idxt[:], in0=idxt[:], in1=gxw[:],
                                        op=ALU.add)

                # ---- E: fold indices to wrapped int16 layout [cq][k][128]
                for k in range(K2):
                    srcT = w9t(idxt)
                    for cq in range(4):
                        pv = ps_tp.tile([16, 128], F32, tag="pidx")
                        nc.tensor.transpose(pv[:], srcT[:, :, cq, k],
                                            identf[:, :])
                        off = (cq * 9 + k) * 128
                        nc.vector.tensor_copy(idxw[0:16, off:off + 128],
                                              pv[:])
                for g in range(1, 8):
                    nc.sync.dma_start(idxw[16 * g:16 * (g + 1), :],
                                      idxw[0:16, :])
                # ---- D: bilinear corner weights + gather indices ----
                basev = rr(base, "p (g s) -> p g s", s=18)

                def v18(t):
                    return rr(t, "p (g s) -> p g s", s=18)

                def v9(t):
                    return rr(t, "p (g s) -> p g s", s=9)

                pypx = tpool.tile([128, 64 * 18], F32)
                nc.vector.tensor_add(out=v18(pypx), in0=omTv[:, :, 0:18],
                                     in1=basev)
                ri32 = tpool.tile([128, 64 * 18], mybir.dt.int32)
                nc.vector.tensor_copy(ri32[:], pypx[:])
                rf32 = tpool.tile([128, 64 * 18], F32)
                nc.vector.tensor_copy(rf32[:], ri32[:])
                gt_ = tpool.tile([128, 64 * 18], F32, tag="ri32")
                nc.vector.tensor_tensor(out=gt_[:], in0=rf32[:], in1=pypx[:],
                                        op=ALU.is_gt)
                flor = tpool.tile([128, 64 * 18], F32)
                nc.vector.tensor_tensor(out=flor[:], in0=rf32[:], in1=gt_[:],
                                        op=ALU.subtract)
                frac = tpool.tile([128, 64 * 18], F32)
                nc.vector.tensor_tensor(out=frac[:], in0=pypx[:], in1=flor[:],
                                        op=ALU.subtract)
                f0c = tpool.tile([128, 64 * 18], F32)
                nc.vector.tensor_scalar(f0c[:], flor[:], SH, 127.0 + SH,
                                        ALU.max, ALU.min)
                v0 = tpool.tile([128, 64 * 18], F32)
                nc.vector.tensor_tensor(out=v0[:], in0=f0c[:], in1=flor[:],
                                        op=ALU.is_equal)
                f1 = tpool.tile([128, 64 * 18], F32)
                nc.vector.tensor_scalar(f1[:], flor[:], 1.0, None, ALU.add)
                f1c = tpool.tile([128, 64 * 18], F32)
                nc.vector.tensor_scalar(f1c[:], f1[:], SH, 127.0 + SH,
                                        ALU.max, ALU.min)
                v1 = tpool.tile([128, 64 * 18], F32)
                nc.vector.tensor_tensor(out=v1[:], in0=f1c[:], in1=f1[:],
                                        op=ALU.is_equal)

                wy, wx = v18(frac)[:, :, 0:9], v18(frac)[:, :, 9:18]
                vy0, vx0 = v18(v0)[:, :, 0:9], v18(v0)[:, :, 9:18]
                vy1, vx1 = v18(v1)[:, :, 0:9], v18(v1)[:, :, 9:18]
                mskf = tpool.tile([128, 64 * 9], F32)
                nc.vector.tensor_copy(mskf[:], omTv[:, :, 18:27])
                msk = v9(mskf)

                a0 = tpool.tile([128, 64 * 9], F32)
                a1 = tpool.tile([128, 64 * 9], F32)
                b0 = tpool.tile([128, 64 * 9], F32)
                b1 = tpool.tile([128, 64 * 9], F32)
                a0v, a1v, b0v, b1v = v9(a0), v9(a1), v9(b0), v9(b1)
                nc.vector.tensor_scalar(a0[:], wy, -1.0, 1.0, ALU.mult, ALU.add)
                nc.vector.tensor_tensor(out=a0v, in0=a0v, in1=vy0, op=ALU.mult)
                nc.vector.tensor_tensor(out=a0v, in0=a0v, in1=msk, op=ALU.mult)
                nc.vector.tensor_tensor(out=a1v, in0=wy, in1=vy1, op=ALU.mult)
                nc.vector.tensor_tensor(out=a1v, in0=a1v, in1=msk, op=ALU.mult)
                nc.vector.tensor_scalar(b0[:], wx, -1.0, 1.0, ALU.mult, ALU.add)
                nc.vector.tensor_tensor(out=b0v, in0=b0v, in1=vx0, op=ALU.mult)
                nc.vector.tensor_tensor(out=b1v, in0=wx, in1=vx1, op=ALU.mult)

                wqv = rr(wq, "p (g k j) -> p g k j", g=64, k=9)
                for j, (ya, xb) in enumerate(((a0v, b0v), (a0v, b1v),
                                              (a1v, b0v), (a1v, b1v))):
                    nc.vector.tensor_tensor(
                        out=wqv[:, :, :, j], in0=ya, in1=xb, op=ALU.mult)

                if DEBUG:
                    nc.sync.dma_start(d_dbg_om[:], om[:])
                    nc.sync.dma_start(d_dbg_omT[:], omT[:])
                    nc.sync.dma_start(d_dbg_omTw[:], omTw[:])
                    nc.sync.dma_start(d_dbg_wq[:], wq[:])
                    nc.sync.dma_start(d_dbg_idxw[:], idxw[:])

            # ---- F/G/H: gather, combine, transpose, main conv ----
            wqv2 = rr(wq, "p (g k j) -> p g k j", g=64, k=9)
            with (
                tc.tile_pool(name="gat", bufs=5) as gpool,
                tc.tile_pool(name="gat1", bufs=2) as gpool1,
                tc.tile_pool(name="outp", bufs=1) as opool,
                tc.tile_pool(name="comb", bufs=2) as mpool,
                tc.tile_pool(name="pstx", bufs=2, space="PSUM") as ps_tx,
                tc.tile_pool(name="psmain", bufs=1, space="PSUM") as ps_main,
            ):
                out_sb = opool.tile([64, R * W], F32, tag="out_sb")
                reg2k = nc.gpsimd.to_reg(2048)
                reg4k = nc.gpsimd.to_reg(4096)
                for cq in range(NQ):
                    ops = ps_main.tile([64, 2048], F32)
                    for pr in range(5):
                        npair = 2 if pr < 4 else 1
                        nslot = GQ * npair
                        gt = (gpool if npair == 2 else gpool1).tile(
                            [128, nslot * 256], BF16, tag=f"gt{npair}")
                        k0 = 2 * pr
                        ioff = (cq * 9 + k0) * 128
                        nc.gpsimd.dma_gather(
                            rr(gt, "p (i e) -> p i e", e=256),
                            d_patch[:],
                            idxw[:, ioff:ioff + 128 * npair],
                            2048 * npair,
                            reg4k if npair == 2 else reg2k,
                            256,
                            single_packet=False,
                            queue_num=(cq * 5 + pr) % 4)
                        # combine per tap: weight-mult + corner reduce
                        if npair == 2:
                            samp2 = mpool.tile([128, GQ * 128], BF16,
                                               tag="s2")
                        else:
                            samp2 = mpool.tile([128, GQ * 64], BF16,
                                               tag="s2s")
                        gtv = rr(gt, "p (i c j) -> p i c j", i=nslot, c=64)
                        for t in range(npair):
                            k = k0 + t
                            tt = mpool.tile([128, GQ * 256], BF16, tag="tt")
                            ttv = rr(tt, "p (g c j) -> p g c j", g=GQ, c=64)
                            nc.vector.tensor_tensor(
                                out=ttv,
                                in0=gtv[:, t * GQ:(t + 1) * GQ, :, :],
                                in1=wqv2[:, 16 * cq:16 * cq + 16, k, None,
                                         :].to_broadcast([128, GQ, 64, 4]),
                                op=ALU.mult)
                            if npair == 2:
                                s2v = rr(samp2, "p (g w2 c) -> p g w2 c",
                                         g=GQ, w2=2)
                                outv = s2v[:, :, t, :]
                            else:
                                outv = rr(samp2, "p (g c) -> p g c", g=GQ)
                            with nc.allow_low_precision(
                                    reason="4-corner bilinear sum, bf16"):
                                nc.vector.tensor_reduce(
                                    out=outv, in_=ttv, axis=AXL.X,
                                    op=ALU.add)
                        # transpose to channel-on-partition
                        cw = 128 if npair == 2 else 64
                        sampT = mpool.tile([cw, GQ * 128], BF16, tag=f"sT{cw}")
                        for half in range(2):
                            px = ps_tx.tile([128, 1024], BF16, tag="px")
                            for j8 in range(8):
                                g16 = half * 8 + j8
                                nc.tensor.transpose(
                                    px[0:cw, j8 * 128:(j8 + 1) * 128],
                                    samp2[:, g16 * cw:(g16 + 1) * cw],
                                    identb[:, :])
                            nc.scalar.copy(
                                sampT[:, half * 1024:(half + 1) * 1024],
                                px[0:cw, :])
                        lhsT = wm2[:, pr * 64:(pr + 1) * 64] if npair == 2 \
                            else wms[:, :]
                        for gb4 in range(4):
                            nc.tensor.matmul(
                                ops[:, gb4 * 512:(gb4 + 1) * 512],
                                lhsT=lhsT,
                                rhs=sampT[:, gb4 * 512:(gb4 + 1) * 512],
                                start=(pr == 0), stop=(pr == 4))
                    nc.scalar.activation(
                        out_sb[:, cq * 2048:(cq + 1) * 2048], ops[:],
                        ACTF.Identity, bias=biast[:, 0:1])
            nc.sync.dma_start(d_out[:], out_sb[:])
    nc.compile()
    return nc


def _prep_core(inputs, core):
    b, r = core // 2, core % 2
    r0 = r * R
    keyt = np.ascontiguousarray(inputs["input_keyt"][b], np.float32)
    inter = np.ascontiguousarray(inputs["inter"][b], np.float32)
    weight = np.asarray(inputs["weight"], np.float32)
    bias = np.asarray(inputs["bias"], np.float32)
    w_om = np.asarray(inputs["w_om"], np.float32)
    b_om = np.asarray(inputs["b_om"], np.float32)

    x2full = np.concatenate([keyt, inter], axis=0)          # (128, 128, 128)
    x2c = np.zeros((128, 66, PW), np.float32)
    lo, hi = max(0, r0 - 1), min(H, r0 + R + 1)
    x2c[:, lo - (r0 - 1):hi - (r0 - 1), 1:129] = x2full[:, lo:hi, :]
    x2 = x2c.reshape(128, -1).astype(BF)

    # 2x2 patch tokens, anchors (y0, x0) in [-1, 128]^2, token [c, (jy, jx)]
    im = keyt.transpose(1, 2, 0)                            # (H, W, C)
    Z = np.zeros((H + 4, W + 4, C), np.float32)
    Z[2:H + 2, 2:W + 2] = im
    # anchor a=y0+1 in [0,130): rows y0+jy = a-1+jy -> Z[a+1+jy]
    P00 = Z[1:1 + PW, 1:1 + PW]
    P01 = Z[1:1 + PW, 2:2 + PW]
    P10 = Z[2:2 + PW, 1:1 + PW]
    P11 = Z[2:2 + PW, 2:2 + PW]
    patch = np.stack([P00, P01, P10, P11], axis=3)          # (130,130,C,4)
    patch = patch.reshape(PW * PW, 4 * C).astype(BF)

    ky = (np.arange(K2) // 3).astype(np.float32)
    kx = (np.arange(K2) % 3).astype(np.float32)
    p_ = np.arange(128, dtype=np.float32)
    g_ = np.arange(64, dtype=np.float32)
    base = np.zeros((128, 64, 18), np.float32)
    base[:, :, 0:9] = (r0 + g_[None, :, None]) - 1 + ky[None, None, :] + SH
    base[:, :, 9:18] = p_[:, None, None] - 1 + kx[None, None, :] + SH

    j_ = np.arange(128)[:, None, None]
    q_ = np.arange(16)[None, :, None]
    c_ = np.arange(4)[None, None, :]
    pg = 16 * (128 * c_ + j_) + q_                          # (128,16,4)
    hl, wl = pg // 128, pg % 128
    basew = np.zeros((128, 16, 4, 18), np.float32)
    basew[:, :, :, 0:9] = (r0 + hl)[..., None] - 1 + ky + SH
    basew[:, :, :, 9:18] = wl[..., None] - 1 + kx + SH

    womt81 = np.zeros((128, 3, 3, 32), np.float32)
    for dy in range(3):
        for dx in range(3):
            womt81[:, dy, dx, 0:27] = w_om[:, :, dy, dx].T
    W9 = weight.reshape(O, C, K2)
    wm2 = np.zeros((128, 4, 64), np.float32)
    for pr in range(4):
        for i in range(2):
            wm2[64 * i:64 * (i + 1), pr, :] = W9[:, :, 2 * pr + i].T
    wms = np.ascontiguousarray(W9[:, :, 8].T)

    return {
        "x2": x2,
        "patch": patch,
        "base": base.reshape(128, -1).astype(BF),
        "basew": basew.reshape(128, -1).astype(BF),
        "womt": womt81.reshape(128, -1).astype(BF),
        "bomt": b_om.reshape(27, 1).astype(np.float32),
        "wm2": wm2.reshape(128, -1).astype(BF),
        "wms": wms.astype(BF),
        "biast": bias.reshape(64, 1).astype(np.float32),
    }


_PROG = None


def kernel(**inputs) -> np.ndarray:
    global _PROG
    from concourse.bass_utils import run_bass_kernel_spmd
    if _PROG is None:
        _PROG = build_program()
    in_maps = [_prep_core(inputs, i) for i in range(NCORES)]
    res = run_bass_kernel_spmd(_PROG, in_maps, core_ids=list(range(NCORES)))
    out = np.zeros((B, O, H, W), np.float32)
    for i in range(NCORES):
        b, r = i // 2, i % 2
        out[b][:, r * R:(r + 1) * R, :] = res.results[i]["out"].reshape(O, R, W)
    return out


# revision 27
# speedup vs baseline: 1.5154x; 1.0015x over previous
"""Modulated deformable conv (DCNv2) Bass kernel for Trainium2, 8 NeuronCores.

Sharding: data-parallel over batch x row-halves; core i handles sample i//2,
output rows 64*(i%2) .. 64*(i%2)+63. No cross-core communication.

v3: multi-queue SWDGE gathers (4 Q7 pairs in parallel), dx-packed offset
conv (81-row lhsT), xbar DMA transposes for both om layouts, and a
2-op-per-tap combine (weight-mult + corner reduce) on a [c, corner] token.

Per-core pipeline:
  B. PE: offset/mask conv (128ch -> 27ch, 3x3) as 3 dy-packed matmuls
     (81-row lhsT = 3 dx shifts), PSUM acc; DVE sums the column-shifted
     dx blocks + bias -> om (bf16, 32 rows).
  C. Xbar DMA transpose: om -> omT natural (position-on-partition) and
     omTw wrapped (gather token order), both [*, 32]-strided bf16.
  D. DVE: fractional bilinear weights with mask + out-of-bounds validity
     folded into 4 corner weights wq[p, g, k, j] (bf16), plus patch-anchor
     gather indices (int32 -> wrapped int16).
  F. Indirect DMA gather (SWDGE): 512B tokens = 64ch x 2x2 pixel patch
     ([c, corner] layout) from a host-prebuilt patch table in HBM; one
     call per (quarter, tap-pair) = 20 calls spread over 4 SWDGE queues
     (4 Q7 descriptor-generator pairs run concurrently).
  G. DVE: per-tap corner-weight multiply (1 op) + corner reduce (1 op).
  H. PE: paired-tap transposes ([128,128] bf16) + main conv with 128-deep
     contraction (2 taps x 64ch); bias added during PSUM->SBUF copy (ACT).
"""
import sys

for _p in ("/opt/trn_rl_repo", "/root/.axon_site/_ro/trn_rl_repo"):
    if _p not in sys.path:
        sys.path.append(_p)

import numpy as np
import ml_dtypes

import concourse.bacc as bacc
import concourse.bass as bass
import concourse.mybir as mybir
import concourse.tile as tile
from concourse.masks import make_identity

F32 = mybir.dt.float32
BF16 = mybir.dt.bfloat16
I32 = mybir.dt.int32
ALU = mybir.AluOpType
ACTF = mybir.ActivationFunctionType
AXL = mybir.AxisListType
BF = ml_dtypes.bfloat16

B, C, H, W = 4, 64, 128, 128
O, K2 = 64, 9
NCORES = 8
R = H // 2             # output rows per core
PW = 130               # patch-table width (anchors -1..128)
NQ = 4                 # quarters of the per-core position space
GQ = 16                # row-chunks (=output rows) per quarter
SH = 4.0               # +4 domain shift so floor domain is positive
KOFF = -(3.0 * PW + 3.0)   # anchor idx = (y0s-3)*130 + (x0s-3)


DEBUG = False


def rr(t, spec, **kw):
    return t[:].rearrange(spec, **kw)


def build_program():
    nc = bacc.Bacc("TRN2", num_swdge_queues=4)
    d_x2 = nc.dram_tensor("x2", [128, 66 * PW], BF16, kind="ExternalInput")
    d_patch = nc.dram_tensor("patch", [PW * PW, 256], BF16, kind="ExternalInput")
    d_base = nc.dram_tensor("base", [128, 64 * 18], BF16, kind="ExternalInput")
    d_basew = nc.dram_tensor("basew", [128, 16 * 4 * 18], BF16, kind="ExternalInput")
    d_womt = nc.dram_tensor("womt", [128, 3 * 96], BF16, kind="ExternalInput")
    d_bomt = nc.dram_tensor("bomt", [27, 1], F32, kind="ExternalInput")
    d_wm2 = nc.dram_tensor("wm2", [128, 4 * 64], BF16, kind="ExternalInput")
    d_wms = nc.dram_tensor("wms", [64, 64], BF16, kind="ExternalInput")
    d_biast = nc.dram_tensor("biast", [64, 1], F32, kind="ExternalInput")
    d_out = nc.dram_tensor("out", [64, R * W], F32, kind="ExternalOutput")
    if DEBUG:
        d_dbg_om = nc.dram_tensor("dbg_om", [32, R * W], BF16,
                                  kind="ExternalOutput")
        d_dbg_omT = nc.dram_tensor("dbg_omT", [128, 64 * 32], BF16,
                                   kind="ExternalOutput")
        d_dbg_omTw = nc.dram_tensor("dbg_omTw", [128, 16 * 4 * 32], BF16,
                                    kind="ExternalOutput")
        d_dbg_wq = nc.dram_tensor("dbg_wq", [128, 64 * 9 * 4], BF16,
                                  kind="ExternalOutput")
        d_dbg_idxw = nc.dram_tensor("dbg_idxw", [128, 9 * 4 * 128],
                                    mybir.dt.int16, kind="ExternalOutput")

    with tile.TileContext(nc) as tc:
        with (
            tc.tile_pool(name="consts", bufs=1) as cpool,
        ):
            identb = cpool.tile([128, 128], BF16)
            make_identity(nc, identb[:])
            identf = cpool.tile([128, 128], F32)
            make_identity(nc, identf[:])
            womt = cpool.tile([128, 3 * 96], BF16)
            bomt = cpool.tile([27, 1], F32)
            base = cpool.tile([128, 64 * 18], BF16)
            basew = cpool.tile([128, 16 * 4 * 18], BF16)
            wm2 = cpool.tile([128, 4 * 64], BF16)
            wms = cpool.tile([64, 64], BF16)
            biast = cpool.tile([64, 1], F32)
            for sb, dr in ((womt, d_womt), (bomt, d_bomt), (base, d_base),
                           (basew, d_basew), (wm2, d_wm2), (wms, d_wms),
                           (biast, d_biast)):
                nc.sync.dma_start(sb[:], dr[:])

            # corner weights wq[p, g64, k9, j4] bf16 + gather indices
            wq = cpool.tile([128, 64 * 9 * 4], BF16)
            idxw = cpool.tile([128, 9 * 4 * 128], mybir.dt.int16)

            with (
                tc.tile_pool(name="mid", bufs=1) as midpool,
                tc.tile_pool(name="tmp", bufs=1) as tpool,
                tc.tile_pool(name="pso", bufs=2, space="PSUM") as ps_om,
                tc.tile_pool(name="pstp", bufs=2, space="PSUM") as ps_tp,
            ):
                omT = midpool.tile([128, 64 * 32], BF16)
                om = midpool.tile([32, R * W], BF16)
                om2 = midpool.tile([32, R * W], BF16)
                omTw = midpool.tile([128, 16 * 4 * 32], BF16)
                omTv = rr(omT, "p (g s) -> p g s", s=32)
                # ---- B: om conv (3 dy-packed matmuls, dx folded on DVE) ----
                x2 = midpool.tile([128, 66 * PW], BF16)
                nc.sync.dma_start(x2[:], d_x2[:])
                x2v = rr(x2, "p (r c) -> p r c", c=PW)
                omv = rr(om, "s (nt c) -> s nt c", c=512)
                om2w = rr(om2, "s (q x) -> s q x", q=16)
                for nt in range(16):
                    ps = ps_om.tile([96, 1024], F32)
                    psf = rr(ps, "p (r x) -> p r x", x=256)
                    for dy in range(3):
                        lhsT = womt[:, dy * 96:(dy + 1) * 96]
                        for r4 in range(4):
                            # start=True clears has_written for the WHOLE
                            # bank: only the first region per bank sets it.
                            nc.tensor.matmul(
                                ps[:, 256 * r4:256 * r4 + 130],
                                lhsT=lhsT,
                                rhs=x2v[:, nt * 4 + dy + r4, :],
                                start=(dy == 0 and r4 % 2 == 0),
                                stop=(dy == 2))
                    t0 = tpool.tile([27, 512], F32, tag="om_t0")
                    t1 = tpool.tile([27, 512], F32, tag="om_t1")
                    nc.vector.tensor_tensor(
                        out=rr(t0, "p (r x) -> p r x", x=128),
                        in0=psf[0:27, :, 0:128],
                        in1=bomt[:, 0:1].to_broadcast([27, 4, 128]), op=ALU.add)
                    nc.vector.tensor_tensor(
                        out=rr(t1, "p (r x) -> p r x", x=128),
                        in0=psf[32:59, :, 1:129],
                        in1=rr(t0, "p (r x) -> p r x", x=128), op=ALU.add)
                    nc.vector.tensor_tensor(
                        out=omv[0:27, nt, :].rearrange("s (r x) -> s r x",
                                                       x=128),
                        in0=psf[64:91, :, 2:130],
                        in1=rr(t1, "p (r x) -> p r x", x=128), op=ALU.add)
                    # wrap-ordered duplicate: om2[s, 512q + xw] = om[s, 16xw + q]
                    nc.vector.tensor_tensor(
                        out=om2w[0:27, :, 32 * nt:32 * nt + 32]
                        .rearrange("s q (r xh) -> s r xh q", xh=8),
                        in0=psf[64:91, :, 2:130]
                        .rearrange("s r (xh q) -> s r xh q", q=16),
                        in1=rr(t1, "p (r xh q) -> p r xh q", xh=8, q=16),
                        op=ALU.add)

                # ---- C: omT natural + omTw wrapped via xbar DMA transpose ----
                nc.sync.dma_start_transpose(omTv, om[:])
                omTwv = rr(omTw, "p (q c s) -> p q c s", q=16, c=4)
                nc.sync.dma_start_transpose(
                    rr(omTw, "p (t s) -> p t s", s=32), om2[:])
                nc.scalar.activation(omTv[:, :, 18:27], omTv[:, :, 18:27],
                                     ACTF.Sigmoid)

                # ---- D2: wrapped pipeline -> patch anchor indices ----
                def w18(t):
                    return rr(t, "p (q c s) -> p q c s", q=16, c=4, s=18)

                def w9t(t):
                    return rr(t, "p (q c s) -> p q c s", q=16, c=4, s=9)

                pypw = tpool.tile([128, 16 * 4 * 18], F32)
                nc.vector.tensor_add(out=w18(pypw),
                                     in0=omTwv[:, :, :, 0:18],
                                     in1=rr(basew, "p (q c s) -> p q c s",
                                            q=16, c=4))
                ri32w = tpool.tile([128, 16 * 4 * 18], mybir.dt.int32)
                nc.vector.tensor_copy(ri32w[:], pypw[:])
                rf32w = tpool.tile([128, 16 * 4 * 18], F32)
                nc.vector.tensor_copy(rf32w[:], ri32w[:])
                gtw_ = tpool.tile([128, 16 * 4 * 18], F32, tag="ri32w")
                nc.vector.tensor_tensor(out=gtw_[:], in0=rf32w[:], in1=pypw[:],
                                        op=ALU.is_gt)
                florw = tpool.tile([128, 16 * 4 * 18], F32)
                nc.vector.tensor_tensor(out=florw[:], in0=rf32w[:],
                                        in1=gtw_[:], op=ALU.subtract)
                # clamp anchors to [-1, 128] (+SH domain: [3, 132])
                fy0w = tpool.tile([128, 16 * 4 * 9], F32)
                nc.vector.tensor_scalar(w9t(fy0w), w18(florw)[:, :, :, 0:9],
                                        SH - 1.0, 128.0 + SH, ALU.max, ALU.min)
                gxw = tpool.tile([128, 16 * 4 * 9], F32)
                nc.vector.tensor_scalar(w9t(gxw), w18(florw)[:, :, :, 9:18],
                                        SH - 1.0, 128.0 + SH, ALU.max, ALU.min)
                idxt = tpool.tile([128, 16 * 4 * 9], F32)
                # anchor = (y0s-3)*130 + (x0s-3), y0s/x0s in +4 domain
                nc.vector.tensor_scalar(idxt[:], fy0w[:], float(PW), KOFF,
                                        ALU.mult, ALU.add)
                nc.vector.tensor_tensor(out=idxt[:], in0=idxt[:], in1=gxw[:],
                                        op=ALU.add)

                # ---- E: fold indices to wrapped int16 layout [cq][k][128]
                for k in range(K2):
                    srcT = w9t(idxt)
                    for cq in range(4):
                        pv = ps_tp.tile([16, 128], F32, tag="pidx")
                        nc.tensor.transpose(pv[:], srcT[:, :, cq, k],
                                            identf[:, :])
                        off = (cq * 9 + k) * 128
                        nc.vector.tensor_copy(idxw[0:16, off:off + 128],
                                              pv[:])
                for g in range(1, 8):
                    nc.sync.dma_start(idxw[16 * g:16 * (g + 1), :],
                                      idxw[0:16, :])
                # ---- D: bilinear corner weights + gather indices ----
                basev = rr(base, "p (g s) -> p g s", s=18)

                def v18(t):
                    return rr(t, "p (g s) -> p g s", s=18)

                def v9(t):
                    return rr(t, "p (g s) -> p g s", s=9)

                pypx = tpool.tile([128, 64 * 18], F32)
                nc.vector.tensor_add(out=v18(pypx), in0=omTv[:, :, 0:18],
                                     in1=basev)
                ri32 = tpool.tile([128, 64 * 18], mybir.dt.int32)
                nc.vector.tensor_copy(ri32[:], pypx[:])
                rf32 = tpool.tile([128, 64 * 18], F32)
                nc.vector.tensor_copy(rf32[:], ri32[:])
                gt_ = tpool.tile([128, 64 * 18], F32, tag="ri32")
                nc.vector.tensor_tensor(out=gt_[:], in0=rf32[:], in1=pypx[:],
                                        op=ALU.is_gt)
                flor = tpool.tile([128, 64 * 18], F32)
                nc.vector.tensor_tensor(out=flor[:], in0=rf32[:], in1=gt_[:],
                                        op=ALU.subtract)
                frac = tpool.tile([128, 64 * 18], F32)
                nc.vector.tensor_tensor(out=frac[:], in0=pypx[:], in1=flor[:],
                                        op=ALU.subtract)
                f0c = tpool.tile([128, 64 * 18], F32)
                nc.vector.tensor_scalar(f0c[:], flor[:], SH, 127.0 + SH,
                                        ALU.max, ALU.min)
                v0 = tpool.tile([128, 64 * 18], F32)
                nc.vector.tensor_tensor(out=v0[:], in0=f0c[:], in1=flor[:],
                                        op=ALU.is_equal)
                f1 = tpool.tile([128, 64 * 18], F32)
                nc.vector.tensor_scalar(f1[:], flor[:], 1.0, None, ALU.add)
                f1c = tpool.tile([128, 64 * 18], F32)
                nc.vector.tensor_scalar(f1c[:], f1[:], SH, 127.0 + SH,
                                        ALU.max, ALU.min)
                v1 = tpool.tile([128, 64 * 18], F32)
                nc.vector.tensor_tensor(out=v1[:], in0=f1c[:], in1=f1[:],
                                        op=ALU.is_equal)

                wy, wx = v18(frac)[:, :, 0:9], v18(frac)[:, :, 9:18]
                vy0, vx0 = v18(v0)[:, :, 0:9], v18(v0)[:, :, 9:18]
                vy1, vx1 = v18(v1)[:, :, 0:9], v18(v1)[:, :, 9:18]
                mskf = tpool.tile([128, 64 * 9], F32)
                nc.vector.tensor_copy(mskf[:], omTv[:, :, 18:27])
                msk = v9(mskf)

                a0 = tpool.tile([128, 64 * 9], F32)
                a1 = tpool.tile([128, 64 * 9], F32)
                b0 = tpool.tile([128, 64 * 9], F32)
                b1 = tpool.tile([128, 64 * 9], F32)
                a0v, a1v, b0v, b1v = v9(a0), v9(a1), v9(b0), v9(b1)
                nc.vector.tensor_scalar(a0[:], wy, -1.0, 1.0, ALU.mult, ALU.add)
                nc.vector.tensor_tensor(out=a0v, in0=a0v, in1=vy0, op=ALU.mult)
                nc.vector.tensor_tensor(out=a0v, in0=a0v, in1=msk, op=ALU.mult)
                nc.vector.tensor_tensor(out=a1v, in0=wy, in1=vy1, op=ALU.mult)
                nc.vector.tensor_tensor(out=a1v, in0=a1v, in1=msk, op=ALU.mult)
                nc.vector.tensor_scalar(b0[:], wx, -1.0, 1.0, ALU.mult, ALU.add)
                nc.vector.tensor_tensor(out=b0v, in0=b0v, in1=vx0, op=ALU.mult)
                nc.vector.tensor_tensor(out=b1v, in0=wx, in1=vx1, op=ALU.mult)

                wqv = rr(wq, "p (g k j) -> p g k j", g=64, k=9)
                for j, (ya, xb) in enumerate(((a0v, b0v), (a0v, b1v),
                                              (a1v, b0v), (a1v, b1v))):
                    nc.vector.tensor_tensor(
                        out=wqv[:, :, :, j], in0=ya, in1=xb, op=ALU.mult)

                if DEBUG:
                    nc.sync.dma_start(d_dbg_om[:], om[:])
                    nc.sync.dma_start(d_dbg_omT[:], omT[:])
                    nc.sync.dma_start(d_dbg_omTw[:], omTw[:])
                    nc.sync.dma_start(d_dbg_wq[:], wq[:])
                    nc.sync.dma_start(d_dbg_idxw[:], idxw[:])

            # ---- F/G/H: gather, combine, transpose, main conv ----
            wqv2 = rr(wq, "p (g k j) -> p g k j", g=64, k=9)
            with (
                tc.tile_pool(name="gat", bufs=5) as gpool,
                tc.tile_pool(name="gat1", bufs=2) as gpool1,
                tc.tile_pool(name="outp", bufs=1) as opool,
                tc.tile_pool(name="comb", bufs=2) as mpool,
                tc.tile_pool(name="pstx", bufs=3, space="PSUM") as ps_tx,
                tc.tile_pool(name="psmain", bufs=1, space="PSUM") as ps_main,
            ):
                out_sb = opool.tile([64, R * W], F32, tag="out_sb")
                reg2k = nc.gpsimd.to_reg(2048)
                reg4k = nc.gpsimd.to_reg(4096)
                for cq in range(NQ):
                    ops = ps_main.tile([64, 2048], F32)
                    for pr in range(5):
                        npair = 2 if pr < 4 else 1
                        nslot = GQ * npair
                        gt = (gpool if npair == 2 else gpool1).tile(
                            [128, nslot * 256], BF16, tag=f"gt{npair}")
                        k0 = 2 * pr
                        ioff = (cq * 9 + k0) * 128
                        nc.gpsimd.dma_gather(
                            rr(gt, "p (i e) -> p i e", e=256),
                            d_patch[:],
                            idxw[:, ioff:ioff + 128 * npair],
                            2048 * npair,
                            reg4k if npair == 2 else reg2k,
                            256,
                            single_packet=False,
                            queue_num=(cq * 5 + pr) % 4)
                        # combine per tap: weight-mult + corner reduce
                        if npair == 2:
                            samp2 = mpool.tile([128, GQ * 128], BF16,
                                               tag="s2")
                        else:
                            samp2 = mpool.tile([128, GQ * 64], BF16,
                                               tag="s2s")
                        gtv = rr(gt, "p (i c j) -> p i c j", i=nslot, c=64)
                        for t in range(npair):
                            k = k0 + t
                            tt = mpool.tile([128, GQ * 256], BF16, tag="tt")
                            ttv = rr(tt, "p (g c j) -> p g c j", g=GQ, c=64)
                            nc.vector.tensor_tensor(
                                out=ttv,
                                in0=gtv[:, t * GQ:(t + 1) * GQ, :, :],
                                in1=wqv2[:, 16 * cq:16 * cq + 16, k, None,
                                         :].to_broadcast([128, GQ, 64, 4]),
                                op=ALU.mult)
                            if npair == 2:
                                s2v = rr(samp2, "p (g w2 c) -> p g w2 c",
                                         g=GQ, w2=2)
                                outv = s2v[:, :, t, :]
                            else:
                                outv = rr(samp2, "p (g c) -> p g c", g=GQ)
                            with nc.allow_low_precision(
                                    reason="4-corner bilinear sum, bf16"):
                                nc.vector.tensor_reduce(
                                    out=outv, in_=ttv, axis=AXL.X,
                                    op=ALU.add)
                        # transpose to channel-on-partition
                        cw = 128 if npair == 2 else 64
                        sampT = mpool.tile([cw, GQ * 128], BF16, tag=f"sT{cw}")
                        for half in range(2):
                            px = ps_tx.tile([128, 1024], BF16, tag="px")
                            for j8 in range(8):
                                g16 = half * 8 + j8
                                nc.tensor.transpose(
                                    px[0:cw, j8 * 128:(j8 + 1) * 128],
                                    samp2[:, g16 * cw:(g16 + 1) * cw],
                                    identb[:, :])
                            nc.scalar.copy(
                                sampT[:, half * 1024:(half + 1) * 1024],
                                px[0:cw, :])
                        lhsT = wm2[:, pr * 64:(pr + 1) * 64] if npair == 2 \
                            else wms[:, :]
                        for gb4 in range(4):
                            nc.tensor.matmul(
                                ops[:, gb4 * 512:(gb4 + 1) * 512],
                                lhsT=lhsT,
                                rhs=sampT[:, gb4 * 512:(gb4 + 1) * 512],
                                start=(pr == 0), stop=(pr == 4))
                    nc.scalar.activation(
                        out_sb[:, cq * 2048:(cq + 1) * 2048], ops[:],
                        ACTF.Identity, bias=biast[:, 0:1])
            nc.sync.dma_start(d_out[:], out_sb[:])
    nc.compile()
    return nc


def _prep_core(inputs, core):
    b, r = core // 2, core % 2
    r0 = r * R
    keyt = np.ascontiguousarray(inputs["input_keyt"][b], np.float32)
    inter = np.ascontiguousarray(inputs["inter"][b], np.float32)
    weight = np.asarray(inputs["weight"], np.float32)
    bias = np.asarray(inputs["bias"], np.float32)
    w_om = np.asarray(inputs["w_om"], np.float32)
    b_om = np.asarray(inputs["b_om"], np.float32)

    x2full = np.concatenate([keyt, inter], axis=0)          # (128, 128, 128)
    x2c = np.zeros((128, 66, PW), np.float32)
    lo, hi = max(0, r0 - 1), min(H, r0 + R + 1)
    x2c[:, lo - (r0 - 1):hi - (r0 - 1), 1:129] = x2full[:, lo:hi, :]
    x2 = x2c.reshape(128, -1).astype(BF)

    # 2x2 patch tokens, anchors (y0, x0) in [-1, 128]^2, token [c, (jy, jx)]
    im = keyt.transpose(1, 2, 0)                            # (H, W, C)
    Z = np.zeros((H + 4, W + 4, C), np.float32)
    Z[2:H + 2, 2:W + 2] = im
    # anchor a=y0+1 in [0,130): rows y0+jy = a-1+jy -> Z[a+1+jy]
    P00 = Z[1:1 + PW, 1:1 + PW]
    P01 = Z[1:1 + PW, 2:2 + PW]
    P10 = Z[2:2 + PW, 1:1 + PW]
    P11 = Z[2:2 + PW, 2:2 + PW]
    patch = np.stack([P00, P01, P10, P11], axis=3)          # (130,130,C,4)
    patch = patch.reshape(PW * PW, 4 * C).astype(BF)

    ky = (np.arange(K2) // 3).astype(np.float32)
    kx = (np.arange(K2) % 3).astype(np.float32)
    p_ = np.arange(128, dtype=np.float32)
    g_ = np.arange(64, dtype=np.float32)
    base = np.zeros((128, 64, 18), np.float32)
    base[:, :, 0:9] = (r0 + g_[None, :, None]) - 1 + ky[None, None, :] + SH
    base[:, :, 9:18] = p_[:, None, None] - 1 + kx[None, None, :] + SH

    j_ = np.arange(128)[:, None, None]
    q_ = np.arange(16)[None, :, None]
    c_ = np.arange(4)[None, None, :]
    pg = 16 * (128 * c_ + j_) + q_                          # (128,16,4)
    hl, wl = pg // 128, pg % 128
    basew = np.zeros((128, 16, 4, 18), np.float32)
    basew[:, :, :, 0:9] = (r0 + hl)[..., None] - 1 + ky + SH
    basew[:, :, :, 9:18] = wl[..., None] - 1 + kx + SH

    womt81 = np.zeros((128, 3, 3, 32), np.float32)
    for dy in range(3):
        for dx in range(3):
            womt81[:, dy, dx, 0:27] = w_om[:, :, dy, dx].T
    W9 = weight.reshape(O, C, K2)
    wm2 = np.zeros((128, 4, 64), np.float32)
    for pr in range(4):
        for i in range(2):
            wm2[64 * i:64 * (i + 1), pr, :] = W9[:, :, 2 * pr + i].T
    wms = np.ascontiguousarray(W9[:, :, 8].T)

    return {
        "x2": x2,
        "patch": patch,
        "base": base.reshape(128, -1).astype(BF),
        "basew": basew.reshape(128, -1).astype(BF),
        "womt": womt81.reshape(128, -1).astype(BF),
        "bomt": b_om.reshape(27, 1).astype(np.float32),
        "wm2": wm2.reshape(128, -1).astype(BF),
        "wms": wms.astype(BF),
        "biast": bias.reshape(64, 1).astype(np.float32),
    }


_PROG = None


def kernel(**inputs) -> np.ndarray:
    global _PROG
    from concourse.bass_utils import run_bass_kernel_spmd
    if _PROG is None:
        _PROG = build_program()
    in_maps = [_prep_core(inputs, i) for i in range(NCORES)]
    res = run_bass_kernel_spmd(_PROG, in_maps, core_ids=list(range(NCORES)))
    out = np.zeros((B, O, H, W), np.float32)
    for i in range(NCORES):
        b, r = i // 2, i % 2
        out[b][:, r * R:(r + 1) * R, :] = res.results[i]["out"].reshape(O, R, W)
    return out


# revision 28
# speedup vs baseline: 1.5233x; 1.0052x over previous
"""Modulated deformable conv (DCNv2) Bass kernel for Trainium2, 8 NeuronCores.

Sharding: data-parallel over batch x row-halves; core i handles sample i//2,
output rows 64*(i%2) .. 64*(i%2)+63. No cross-core communication.

v3: multi-queue SWDGE gathers (4 Q7 pairs in parallel), dx-packed offset
conv (81-row lhsT), xbar DMA transposes for both om layouts, and a
2-op-per-tap combine (weight-mult + corner reduce) on a [c, corner] token.

Per-core pipeline:
  B. PE: offset/mask conv (128ch -> 27ch, 3x3) as 3 dy-packed matmuls
     (81-row lhsT = 3 dx shifts), PSUM acc; DVE sums the column-shifted
     dx blocks + bias -> om (bf16, 32 rows).
  C. Xbar DMA transpose: om -> omT natural (position-on-partition) and
     omTw wrapped (gather token order), both [*, 32]-strided bf16.
  D. DVE: fractional bilinear weights with mask + out-of-bounds validity
     folded into 4 corner weights wq[p, g, k, j] (bf16), plus patch-anchor
     gather indices (int32 -> wrapped int16).
  F. Indirect DMA gather (SWDGE): 512B tokens = 64ch x 2x2 pixel patch
     ([c, corner] layout) from a host-prebuilt patch table in HBM; one
     call per (quarter, tap-pair) = 20 calls spread over 4 SWDGE queues
     (4 Q7 descriptor-generator pairs run concurrently).
  G. DVE: per-tap corner-weight multiply (1 op) + corner reduce (1 op).
  H. PE: paired-tap transposes ([128,128] bf16) + main conv with 128-deep
     contraction (2 taps x 64ch); bias added during PSUM->SBUF copy (ACT).
"""
import sys

for _p in ("/opt/trn_rl_repo", "/root/.axon_site/_ro/trn_rl_repo"):
    if _p not in sys.path:
        sys.path.append(_p)

import numpy as np
import ml_dtypes

import concourse.bacc as bacc
import concourse.bass as bass
import concourse.mybir as mybir
import concourse.tile as tile
from concourse.masks import make_identity

F32 = mybir.dt.float32
BF16 = mybir.dt.bfloat16
I32 = mybir.dt.int32
ALU = mybir.AluOpType
ACTF = mybir.ActivationFunctionType
AXL = mybir.AxisListType
BF = ml_dtypes.bfloat16

B, C, H, W = 4, 64, 128, 128
O, K2 = 64, 9
NCORES = 8
R = H // 2             # output rows per core
PW = 130               # patch-table width (anchors -1..128)
NQ = 4                 # quarters of the per-core position space
GQ = 16                # row-chunks (=output rows) per quarter
SH = 4.0               # +4 domain shift so floor domain is positive
KOFF = -(3.0 * PW + 3.0)   # anchor idx = (y0s-3)*130 + (x0s-3)


DEBUG = False


def rr(t, spec, **kw):
    return t[:].rearrange(spec, **kw)


def build_program():
    nc = bacc.Bacc("TRN2", num_swdge_queues=4)
    d_x2 = nc.dram_tensor("x2", [128, 66 * PW], BF16, kind="ExternalInput")
    d_patch = nc.dram_tensor("patch", [PW * PW, 256], BF16, kind="ExternalInput")
    d_base = nc.dram_tensor("base", [128, 64 * 18], BF16, kind="ExternalInput")
    d_basew = nc.dram_tensor("basew", [128, 16 * 4 * 18], BF16, kind="ExternalInput")
    d_womt = nc.dram_tensor("womt", [128, 3 * 96], BF16, kind="ExternalInput")
    d_bomt = nc.dram_tensor("bomt", [27, 1], F32, kind="ExternalInput")
    d_wm2 = nc.dram_tensor("wm2", [128, 4 * 64], BF16, kind="ExternalInput")
    d_wms = nc.dram_tensor("wms", [64, 64], BF16, kind="ExternalInput")
    d_biast = nc.dram_tensor("biast", [64, 1], F32, kind="ExternalInput")
    d_out = nc.dram_tensor("out", [64, R * W], F32, kind="ExternalOutput")
    if DEBUG:
        d_dbg_om = nc.dram_tensor("dbg_om", [32, R * W], BF16,
                                  kind="ExternalOutput")
        d_dbg_omT = nc.dram_tensor("dbg_omT", [128, 64 * 32], BF16,
                                   kind="ExternalOutput")
        d_dbg_omTw = nc.dram_tensor("dbg_omTw", [128, 16 * 4 * 32], BF16,
                                    kind="ExternalOutput")
        d_dbg_wq = nc.dram_tensor("dbg_wq", [128, 64 * 9 * 4], BF16,
                                  kind="ExternalOutput")
        d_dbg_idxw = nc.dram_tensor("dbg_idxw", [128, 9 * 4 * 128],
                                    mybir.dt.int16, kind="ExternalOutput")

    with tile.TileContext(nc) as tc:
        with (
            tc.tile_pool(name="consts", bufs=1) as cpool,
        ):
            identb = cpool.tile([128, 128], BF16)
            make_identity(nc, identb[:])
            identf = cpool.tile([128, 128], F32)
            make_identity(nc, identf[:])
            womt = cpool.tile([128, 3 * 96], BF16)
            bomt = cpool.tile([27, 1], F32)
            base = cpool.tile([128, 64 * 18], BF16)
            basew = cpool.tile([128, 16 * 4 * 18], BF16)
            wm2 = cpool.tile([128, 4 * 64], BF16)
            wms = cpool.tile([64, 64], BF16)
            biast = cpool.tile([64, 1], F32)
            for sb, dr in ((womt, d_womt), (bomt, d_bomt), (base, d_base),
                           (basew, d_basew), (wm2, d_wm2), (wms, d_wms),
                           (biast, d_biast)):
                nc.sync.dma_start(sb[:], dr[:])

            # corner weights wq[p, g64, k9, j4] bf16 + gather indices
            wq = cpool.tile([128, 64 * 9 * 4], BF16)
            idxw = cpool.tile([128, 9 * 4 * 128], mybir.dt.int16)

            with (
                tc.tile_pool(name="mid", bufs=1) as midpool,
                tc.tile_pool(name="tmp", bufs=1) as tpool,
                tc.tile_pool(name="pso", bufs=3, space="PSUM") as ps_om,
                tc.tile_pool(name="pstp", bufs=2, space="PSUM") as ps_tp,
            ):
                omT = midpool.tile([128, 64 * 32], BF16)
                om = midpool.tile([32, R * W], BF16)
                om2 = midpool.tile([32, R * W], BF16)
                omTw = midpool.tile([128, 16 * 4 * 32], BF16)
                omTv = rr(omT, "p (g s) -> p g s", s=32)
                # ---- B: om conv (3 dy-packed matmuls, dx folded on DVE) ----
                x2 = midpool.tile([128, 66 * PW], BF16)
                nc.sync.dma_start(x2[:], d_x2[:])
                x2v = rr(x2, "p (r c) -> p r c", c=PW)
                omv = rr(om, "s (nt c) -> s nt c", c=512)
                om2w = rr(om2, "s (q x) -> s q x", q=16)
                for nt in range(16):
                    ps = ps_om.tile([96, 1024], F32)
                    psf = rr(ps, "p (r x) -> p r x", x=256)
                    for dy in range(3):
                        lhsT = womt[:, dy * 96:(dy + 1) * 96]
                        for r4 in range(4):
                            # start=True clears has_written for the WHOLE
                            # bank: only the first region per bank sets it.
                            nc.tensor.matmul(
                                ps[:, 256 * r4:256 * r4 + 130],
                                lhsT=lhsT,
                                rhs=x2v[:, nt * 4 + dy + r4, :],
                                start=(dy == 0 and r4 % 2 == 0),
                                stop=(dy == 2))
                    t0 = tpool.tile([27, 512], F32, tag="om_t0")
                    t1 = tpool.tile([27, 512], F32, tag="om_t1")
                    nc.vector.tensor_tensor(
                        out=rr(t0, "p (r x) -> p r x", x=128),
                        in0=psf[0:27, :, 0:128],
                        in1=bomt[:, 0:1].to_broadcast([27, 4, 128]), op=ALU.add)
                    nc.vector.tensor_tensor(
                        out=rr(t1, "p (r x) -> p r x", x=128),
                        in0=psf[32:59, :, 1:129],
                        in1=rr(t0, "p (r x) -> p r x", x=128), op=ALU.add)
                    nc.vector.tensor_tensor(
                        out=omv[0:27, nt, :].rearrange("s (r x) -> s r x",
                                                       x=128),
                        in0=psf[64:91, :, 2:130],
                        in1=rr(t1, "p (r x) -> p r x", x=128), op=ALU.add)
                    # wrap-ordered duplicate: om2[s, 512q + xw] = om[s, 16xw + q]
                    nc.vector.tensor_tensor(
                        out=om2w[0:27, :, 32 * nt:32 * nt + 32]
                        .rearrange("s q (r xh) -> s r xh q", xh=8),
                        in0=psf[64:91, :, 2:130]
                        .rearrange("s r (xh q) -> s r xh q", q=16),
                        in1=rr(t1, "p (r xh q) -> p r xh q", xh=8, q=16),
                        op=ALU.add)

                # ---- C: omT natural + omTw wrapped via xbar DMA transpose ----
                nc.sync.dma_start_transpose(omTv, om[:])
                omTwv = rr(omTw, "p (q c s) -> p q c s", q=16, c=4)
                nc.sync.dma_start_transpose(
                    rr(omTw, "p (t s) -> p t s", s=32), om2[:])
                nc.scalar.activation(omTv[:, :, 18:27], omTv[:, :, 18:27],
                                     ACTF.Sigmoid)

                # ---- D2: wrapped pipeline -> patch anchor indices ----
                def w18(t):
                    return rr(t, "p (q c s) -> p q c s", q=16, c=4, s=18)

                def w9t(t):
                    return rr(t, "p (q c s) -> p q c s", q=16, c=4, s=9)

                pypw = tpool.tile([128, 16 * 4 * 18], F32)
                nc.vector.tensor_add(out=w18(pypw),
                                     in0=omTwv[:, :, :, 0:18],
                                     in1=rr(basew, "p (q c s) -> p q c s",
                                            q=16, c=4))
                ri32w = tpool.tile([128, 16 * 4 * 18], mybir.dt.int32)
                nc.vector.tensor_copy(ri32w[:], pypw[:])
                rf32w = tpool.tile([128, 16 * 4 * 18], F32)
                nc.vector.tensor_copy(rf32w[:], ri32w[:])
                gtw_ = tpool.tile([128, 16 * 4 * 18], F32, tag="ri32w")
                nc.vector.tensor_tensor(out=gtw_[:], in0=rf32w[:], in1=pypw[:],
                                        op=ALU.is_gt)
                florw = tpool.tile([128, 16 * 4 * 18], F32)
                nc.vector.tensor_tensor(out=florw[:], in0=rf32w[:],
                                        in1=gtw_[:], op=ALU.subtract)
                # clamp anchors to [-1, 128] (+SH domain: [3, 132])
                fy0w = tpool.tile([128, 16 * 4 * 9], F32)
                nc.vector.tensor_scalar(w9t(fy0w), w18(florw)[:, :, :, 0:9],
                                        SH - 1.0, 128.0 + SH, ALU.max, ALU.min)
                gxw = tpool.tile([128, 16 * 4 * 9], F32)
                nc.vector.tensor_scalar(w9t(gxw), w18(florw)[:, :, :, 9:18],
                                        SH - 1.0, 128.0 + SH, ALU.max, ALU.min)
                idxt = tpool.tile([128, 16 * 4 * 9], F32)
                # anchor = (y0s-3)*130 + (x0s-3), y0s/x0s in +4 domain
                nc.vector.tensor_scalar(idxt[:], fy0w[:], float(PW), KOFF,
                                        ALU.mult, ALU.add)
                nc.vector.tensor_tensor(out=idxt[:], in0=idxt[:], in1=gxw[:],
                                        op=ALU.add)

                # ---- E: fold indices to wrapped int16 layout [cq][k][128]
                for k in range(K2):
                    srcT = w9t(idxt)
                    for cq in range(4):
                        pv = ps_tp.tile([16, 128], F32, tag="pidx")
                        nc.tensor.transpose(pv[:], srcT[:, :, cq, k],
                                            identf[:, :])
                        off = (cq * 9 + k) * 128
                        nc.vector.tensor_copy(idxw[0:16, off:off + 128],
                                              pv[:])
                for g in range(1, 8):
                    nc.sync.dma_start(idxw[16 * g:16 * (g + 1), :],
                                      idxw[0:16, :])
                # ---- D: bilinear corner weights + gather indices ----
                basev = rr(base, "p (g s) -> p g s", s=18)

                def v18(t):
                    return rr(t, "p (g s) -> p g s", s=18)

                def v9(t):
                    return rr(t, "p (g s) -> p g s", s=9)

                pypx = tpool.tile([128, 64 * 18], F32)
                nc.vector.tensor_add(out=v18(pypx), in0=omTv[:, :, 0:18],
                                     in1=basev)
                ri32 = tpool.tile([128, 64 * 18], mybir.dt.int32)
                nc.vector.tensor_copy(ri32[:], pypx[:])
                rf32 = tpool.tile([128, 64 * 18], F32)
                nc.vector.tensor_copy(rf32[:], ri32[:])
                gt_ = tpool.tile([128, 64 * 18], F32, tag="ri32")
                nc.vector.tensor_tensor(out=gt_[:], in0=rf32[:], in1=pypx[:],
                                        op=ALU.is_gt)
                flor = tpool.tile([128, 64 * 18], F32)
                nc.vector.tensor_tensor(out=flor[:], in0=rf32[:], in1=gt_[:],
                                        op=ALU.subtract)
                frac = tpool.tile([128, 64 * 18], F32)
                nc.vector.tensor_tensor(out=frac[:], in0=pypx[:], in1=flor[:],
                                        op=ALU.subtract)
                f0c = tpool.tile([128, 64 * 18], F32)
                nc.vector.tensor_scalar(f0c[:], flor[:], SH, 127.0 + SH,
                                        ALU.max, ALU.min)
                v0 = tpool.tile([128, 64 * 18], F32)
                nc.vector.tensor_tensor(out=v0[:], in0=f0c[:], in1=flor[:],
                                        op=ALU.is_equal)
                f1 = tpool.tile([128, 64 * 18], F32)
                nc.vector.tensor_scalar(f1[:], flor[:], 1.0, None, ALU.add)
                f1c = tpool.tile([128, 64 * 18], F32)
                nc.vector.tensor_scalar(f1c[:], f1[:], SH, 127.0 + SH,
                                        ALU.max, ALU.min)
                v1 = tpool.tile([128, 64 * 18], F32)
                nc.vector.tensor_tensor(out=v1[:], in0=f1c[:], in1=f1[:],
                                        op=ALU.is_equal)

                wy, wx = v18(frac)[:, :, 0:9], v18(frac)[:, :, 9:18]
                vy0, vx0 = v18(v0)[:, :, 0:9], v18(v0)[:, :, 9:18]
                vy1, vx1 = v18(v1)[:, :, 0:9], v18(v1)[:, :, 9:18]
                mskf = tpool.tile([128, 64 * 9], F32)
                nc.vector.tensor_copy(mskf[:], omTv[:, :, 18:27])
                msk = v9(mskf)

                a0 = tpool.tile([128, 64 * 9], F32)
                a1 = tpool.tile([128, 64 * 9], F32)
                b0 = tpool.tile([128, 64 * 9], F32)
                b1 = tpool.tile([128, 64 * 9], F32)
                a0v, a1v, b0v, b1v = v9(a0), v9(a1), v9(b0), v9(b1)
                nc.vector.tensor_scalar(a0[:], wy, -1.0, 1.0, ALU.mult, ALU.add)
                nc.vector.tensor_tensor(out=a0v, in0=a0v, in1=vy0, op=ALU.mult)
                nc.vector.tensor_tensor(out=a0v, in0=a0v, in1=msk, op=ALU.mult)
                nc.vector.tensor_tensor(out=a1v, in0=wy, in1=vy1, op=ALU.mult)
                nc.vector.tensor_tensor(out=a1v, in0=a1v, in1=msk, op=ALU.mult)
                nc.vector.tensor_scalar(b0[:], wx, -1.0, 1.0, ALU.mult, ALU.add)
                nc.vector.tensor_tensor(out=b0v, in0=b0v, in1=vx0, op=ALU.mult)
                nc.vector.tensor_tensor(out=b1v, in0=wx, in1=vx1, op=ALU.mult)

                wqv = rr(wq, "p (g k j) -> p g k j", g=64, k=9)
                for j, (ya, xb) in enumerate(((a0v, b0v), (a0v, b1v),
                                              (a1v, b0v), (a1v, b1v))):
                    nc.vector.tensor_tensor(
                        out=wqv[:, :, :, j], in0=ya, in1=xb, op=ALU.mult)

                if DEBUG:
                    nc.sync.dma_start(d_dbg_om[:], om[:])
                    nc.sync.dma_start(d_dbg_omT[:], omT[:])
                    nc.sync.dma_start(d_dbg_omTw[:], omTw[:])
                    nc.sync.dma_start(d_dbg_wq[:], wq[:])
                    nc.sync.dma_start(d_dbg_idxw[:], idxw[:])

            # ---- F/G/H: gather, combine, transpose, main conv ----
            wqv2 = rr(wq, "p (g k j) -> p g k j", g=64, k=9)
            with (
                tc.tile_pool(name="gat", bufs=5) as gpool,
                tc.tile_pool(name="gat1", bufs=3) as gpool1,
                tc.tile_pool(name="outp", bufs=1) as opool,
                tc.tile_pool(name="comb", bufs=2) as mpool,
                tc.tile_pool(name="pstx", bufs=3, space="PSUM") as ps_tx,
                tc.tile_pool(name="psmain", bufs=1, space="PSUM") as ps_main,
            ):
                out_sb = opool.tile([64, R * W], F32, tag="out_sb")
                reg2k = nc.gpsimd.to_reg(2048)
                reg4k = nc.gpsimd.to_reg(4096)
                for cq in range(NQ):
                    ops = ps_main.tile([64, 2048], F32)
                    for pr in range(5):
                        npair = 2 if pr < 4 else 1
                        nslot = GQ * npair
                        gt = (gpool if npair == 2 else gpool1).tile(
                            [128, nslot * 256], BF16, tag=f"gt{npair}")
                        k0 = 2 * pr
                        ioff = (cq * 9 + k0) * 128
                        nc.gpsimd.dma_gather(
                            rr(gt, "p (i e) -> p i e", e=256),
                            d_patch[:],
                            idxw[:, ioff:ioff + 128 * npair],
                            2048 * npair,
                            reg4k if npair == 2 else reg2k,
                            256,
                            single_packet=False,
                            queue_num=(cq * 5 + pr) % 4)
                        # combine per tap: weight-mult + corner reduce
                        if npair == 2:
                            samp2 = mpool.tile([128, GQ * 128], BF16,
                                               tag="s2")
                        else:
                            samp2 = mpool.tile([128, GQ * 64], BF16,
                                               tag="s2s")
                        gtv = rr(gt, "p (i c j) -> p i c j", i=nslot, c=64)
                        for t in range(npair):
                            k = k0 + t
                            tt = mpool.tile([128, GQ * 256], BF16, tag="tt")
                            ttv = rr(tt, "p (g c j) -> p g c j", g=GQ, c=64)
                            nc.vector.tensor_tensor(
                                out=ttv,
                                in0=gtv[:, t * GQ:(t + 1) * GQ, :, :],
                                in1=wqv2[:, 16 * cq:16 * cq + 16, k, None,
                                         :].to_broadcast([128, GQ, 64, 4]),
                                op=ALU.mult)
                            if npair == 2:
                                s2v = rr(samp2, "p (g w2 c) -> p g w2 c",
                                         g=GQ, w2=2)
                                outv = s2v[:, :, t, :]
                            else:
                                outv = rr(samp2, "p (g c) -> p g c", g=GQ)
                            with nc.allow_low_precision(
                                    reason="4-corner bilinear sum, bf16"):
                                nc.vector.tensor_reduce(
                                    out=outv, in_=ttv, axis=AXL.X,
                                    op=ALU.add)
                        # transpose to channel-on-partition
                        cw = 128 if npair == 2 else 64
                        sampT = mpool.tile([cw, GQ * 128], BF16, tag=f"sT{cw}")
                        for half in range(2):
                            px = ps_tx.tile([128, 1024], BF16, tag="px")
                            for j8 in range(8):
                                g16 = half * 8 + j8
                                nc.tensor.transpose(
                                    px[0:cw, j8 * 128:(j8 + 1) * 128],
                                    samp2[:, g16 * cw:(g16 + 1) * cw],
                                    identb[:, :])
                            nc.scalar.copy(
                                sampT[:, half * 1024:(half + 1) * 1024],
                                px[0:cw, :])
                        lhsT = wm2[:, pr * 64:(pr + 1) * 64] if npair == 2 \
                            else wms[:, :]
                        for gb4 in range(4):
                            nc.tensor.matmul(
                                ops[:, gb4 * 512:(gb4 + 1) * 512],
                                lhsT=lhsT,
                                rhs=sampT[:, gb4 * 512:(gb4 + 1) * 512],
                                start=(pr == 0), stop=(pr == 4))
                    nc.scalar.activation(
                        out_sb[:, cq * 2048:(cq + 1) * 2048], ops[:],
                        ACTF.Identity, bias=biast[:, 0:1])
            nc.sync.dma_start(d_out[:], out_sb[:])
    nc.compile()
    return nc


def _prep_core(inputs, core):
    b, r = core // 2, core % 2
    r0 = r * R
    keyt = np.ascontiguousarray(inputs["input_keyt"][b], np.float32)
    inter = np.ascontiguousarray(inputs["inter"][b], np.float32)
    weight = np.asarray(inputs["weight"], np.float32)
    bias = np.asarray(inputs["bias"], np.float32)
    w_om = np.asarray(inputs["w_om"], np.float32)
    b_om = np.asarray(inputs["b_om"], np.float32)

    x2full = np.concatenate([keyt, inter], axis=0)          # (128, 128, 128)
    x2c = np.zeros((128, 66, PW), np.float32)
    lo, hi = max(0, r0 - 1), min(H, r0 + R + 1)
    x2c[:, lo - (r0 - 1):hi - (r0 - 1), 1:129] = x2full[:, lo:hi, :]
    x2 = x2c.reshape(128, -1).astype(BF)

    # 2x2 patch tokens, anchors (y0, x0) in [-1, 128]^2, token [c, (jy, jx)]
    im = keyt.transpose(1, 2, 0)                            # (H, W, C)
    Z = np.zeros((H + 4, W + 4, C), np.float32)
    Z[2:H + 2, 2:W + 2] = im
    # anchor a=y0+1 in [0,130): rows y0+jy = a-1+jy -> Z[a+1+jy]
    P00 = Z[1:1 + PW, 1:1 + PW]
    P01 = Z[1:1 + PW, 2:2 + PW]
    P10 = Z[2:2 + PW, 1:1 + PW]
    P11 = Z[2:2 + PW, 2:2 + PW]
    patch = np.stack([P00, P01, P10, P11], axis=3)          # (130,130,C,4)
    patch = patch.reshape(PW * PW, 4 * C).astype(BF)

    ky = (np.arange(K2) // 3).astype(np.float32)
    kx = (np.arange(K2) % 3).astype(np.float32)
    p_ = np.arange(128, dtype=np.float32)
    g_ = np.arange(64, dtype=np.float32)
    base = np.zeros((128, 64, 18), np.float32)
    base[:, :, 0:9] = (r0 + g_[None, :, None]) - 1 + ky[None, None, :] + SH
    base[:, :, 9:18] = p_[:, None, None] - 1 + kx[None, None, :] + SH

    j_ = np.arange(128)[:, None, None]
    q_ = np.arange(16)[None, :, None]
    c_ = np.arange(4)[None, None, :]
    pg = 16 * (128 * c_ + j_) + q_                          # (128,16,4)
    hl, wl = pg // 128, pg % 128
    basew = np.zeros((128, 16, 4, 18), np.float32)
    basew[:, :, :, 0:9] = (r0 + hl)[..., None] - 1 + ky + SH
    basew[:, :, :, 9:18] = wl[..., None] - 1 + kx + SH

    womt81 = np.zeros((128, 3, 3, 32), np.float32)
    for dy in range(3):
        for dx in range(3):
            womt81[:, dy, dx, 0:27] = w_om[:, :, dy, dx].T
    W9 = weight.reshape(O, C, K2)
    wm2 = np.zeros((128, 4, 64), np.float32)
    for pr in range(4):
        for i in range(2):
            wm2[64 * i:64 * (i + 1), pr, :] = W9[:, :, 2 * pr + i].T
    wms = np.ascontiguousarray(W9[:, :, 8].T)

    return {
        "x2": x2,
        "patch": patch,
        "base": base.reshape(128, -1).astype(BF),
        "basew": basew.reshape(128, -1).astype(BF),
        "womt": womt81.reshape(128, -1).astype(BF),
        "bomt": b_om.reshape(27, 1).astype(np.float32),
        "wm2": wm2.reshape(128, -1).astype(BF),
        "wms": wms.astype(BF),
        "biast": bias.reshape(64, 1).astype(np.float32),
    }


_PROG = None


def kernel(**inputs) -> np.ndarray:
    global _PROG
    from concourse.bass_utils import run_bass_kernel_spmd
    if _PROG is None:
        _PROG = build_program()
    in_maps = [_prep_core(inputs, i) for i in range(NCORES)]
    res = run_bass_kernel_spmd(_PROG, in_maps, core_ids=list(range(NCORES)))
    out = np.zeros((B, O, H, W), np.float32)
    for i in range(NCORES):
        b, r = i // 2, i % 2
        out[b][:, r * R:(r + 1) * R, :] = res.results[i]["out"].reshape(O, R, W)
    return out
